# revision 1
# baseline (speedup 1.0000x reference)
"""Multi-head causal self-attention (B=4, T=2048, C=1024, 16 heads) on 8 trn2 cores.

Sharding: data-parallel over batch (4) x tensor-parallel over heads (2 groups of 8).
Core m handles batch m//2, head group m%2. Host pre-transposes x and the weights so
every on-device matmul consumes operands in natural layout (zero on-device
transposes); the output projection partial sums are pair-reduced on host (+bias).

Per-core pipeline (all matmuls fp32r = FP22 multiply, fp32 PSUM accumulate):
  qT[o,t] = Wq_g @ x^T        (lhsT = Wq_g^T chunks, rhs = x^T chunks)
  kT[o,t] likewise; v[t,o]    (lhsT = x^T chunks, rhs = Wv_g^T)
  scores^T[k,q] per head      (lhsT = kT tile [64,128], rhs = qT tile [64,512])
  p = exp(0.125*scores^T)     (ACT, causal mask via memset + triangle multiply)
  [AV^T | denom] = [v|1]^T @ p (ones column of v gives softmax denominators)
  avT = AV^T * exp(-ln(denom)) broadcast via K=1 ones-matmul
  out_partial = avT^T @ Wp_g^T
"""

import numpy as np

import concourse.bass as bass
import concourse.mybir as mybir
import concourse.tile as tile
from concourse.bass_utils import run_bass_kernel_spmd

F32 = mybir.dt.float32
F32R = mybir.dt.float32r
AF = mybir.ActivationFunctionType
MULT = mybir.AluOpType.mult

B, T, C = 4, 2048, 1024
HEADS, D = 16, 64
GROUPS = 2                  # head groups (tensor parallel)
HPC = HEADS // GROUPS       # heads per core = 8
GC = HPC * D                # group channel width = 512
NKC = T // 128              # Tk chunks = 16
NJ = T // 512               # Tq tiles = 4
CCH = C // 128              # contraction chunks = 8
NSTRIP = T // 512           # phase-1 t strips = 4

_PROGRAM = None


def _patch_drain_chunking():
    """The axon walrus build rejects instructions with >~4 sem waits; Tile's
    kernel-tail drain waits on every live semaphore at once. Split it into a
    chain of drains with <=2 waits each."""
    from bass_rust import VectorClock, ScopedClock

    if getattr(tile.TileContext, "_drain_chunk_patched", False):
        return

    def _drain_and_barrier(self, tick_clock, wait_clock):
        gc_vec = list(tick_clock.global_clock)
        nz = [i for i, t in enumerate(gc_vec) if t > 0]
        CHUNK = 1
        for k in range(0, len(nz), CHUNK):
            keep = set(nz[k:k + CHUNK])
            partial = [gc_vec[i] if i in keep else 0 for i in range(len(gc_vec))]
            d = self.nc.sync.drain()
            wait_clock.add_sem_waits(d.ins, ScopedClock({None: VectorClock(partial)}))
        self.nc.all_engine_barrier()
        assert self.sems is not None
        popped = self.nc._tile_sem_poison_stack.pop()
        assert popped is self._sem_poison
        self.nc.clear_and_free_semaphores(list(self.sems.allocated().values()))
        self.nc.all_engine_barrier()

    tile.TileContext._drain_and_barrier = _drain_and_barrier
    tile.TileContext._drain_chunk_patched = True


def _split_excess_waits(nc, maxw=1, maxw_other=None):
    """Walrus rejects instructions carrying more than ~1 sem wait (proven for
    PE matmul S3_LW and the SP drain at 5). Move excess waits onto same-engine
    NoOps inserted immediately before the instruction (engine streams execute
    in bb order, so semantics are preserved). maxw_other, if set, applies to
    non-PE engines."""
    from bass_rust import InstNoOp

    ctr = 0
    for f in nc.m.functions:
        for bb in f.blocks:
            new_insts = []
            for inst in bb.instructions:
                si = inst.sync_info
                waits = list(si.on_wait) if si and si.on_wait else []
                lim = maxw
                if maxw_other is not None and str(inst.engine) != 'EngineType.PE':
                    lim = maxw_other
                maxw_eff = lim
                if len(waits) > maxw_eff:
                    head, rest = waits[:-maxw_eff], waits[-maxw_eff:]
                    for k in range(0, len(head), maxw_eff):
                        ctr += 1
                        new_insts.append(InstNoOp(
                            name=f"waitnop_{ctr}",
                            engine=inst.engine,
                            sync_info=mybir.SyncInfo(
                                on_wait=head[k:k + maxw_eff], on_update=[]),
                        ))
                    inst.sync_info = mybir.SyncInfo(on_wait=rest, on_update=si.on_update)
                new_insts.append(inst)
            bb.instructions = new_insts
    return ctr


def _build_program():
    _patch_drain_chunking()
    nc = bass.Bass()

    xT_d = nc.declare_dram_parameter("xT", [C, T], F32R, isOutput=False)
    wq_d = nc.declare_dram_parameter("wqT", [C, GC], F32R, isOutput=False)
    wk_d = nc.declare_dram_parameter("wkT", [C, GC], F32R, isOutput=False)
    wv_d = nc.declare_dram_parameter("wvT", [C, GC], F32R, isOutput=False)
    wp_d = nc.declare_dram_parameter("wpT", [GC, C], F32R, isOutput=False)
    out_d = nc.declare_dram_parameter("outp", [T, C], F32, isOutput=True)

    from contextlib import ExitStack

    with tile.TileContext(nc) as tc, ExitStack() as stack:
        cpool = stack.enter_context(tc.tile_pool(name="const", bufs=1))
        qkv_pool = stack.enter_context(tc.tile_pool(name="qkv", bufs=1))

        # additive causal mask: 0 where q >= k, -1e9 where q < k (exp -> 0)
        maskneg = cpool.tile([128, 128], F32)
        nc.gpsimd.memset(maskneg[:, :], 0.0)
        nc.gpsimd.affine_select(
            out=maskneg[:, :], in_=maskneg[:, :],
            compare_op=mybir.AluOpType.is_ge, fill=-1e9, base=0,
            pattern=[[1, 128]], channel_multiplier=-1,
        )
        # DVE cannot encode f32r, so f32r tiles are written by ACT/DMA only
        ones = cpool.tile([128, 128], F32R)
        nc.scalar.activation(ones[64:65, :], ones[64:65, :], AF.Copy, scale=0.0, bias=1.0)

        qT = qkv_pool.tile([128, HPC // 2, T], F32R)   # [c, head-pair, t]
        kT = qkv_pool.tile([128, HPC // 2, T], F32R)
        # v padded with a ones column per head: [t-chunk, head, 65]
        v = qkv_pool.tile([128, NKC, HPC, D + 1], F32R)
        nc.scalar.activation(v[:, :, :, D:D + 1], v[:, :, :, D:D + 1],
                             AF.Copy, scale=0.0, bias=1.0)

        # ---------------- Phase 1: QKV projections ----------------
        with tc.tile_pool(name="w1", bufs=1) as wpool, \
             tc.tile_pool(name="xs", bufs=3) as xpool, \
             tc.tile_pool(name="tmp1", bufs=4) as tmp1, \
             tc.tile_pool(name="ps1", bufs=8, space="PSUM") as ps1:
            wq = wpool.tile([128, CCH, GC], F32R)
            wk = wpool.tile([128, CCH, GC], F32R)
            wv = wpool.tile([128, CCH, GC], F32R)

            for s in range(NSTRIP):
                xs = xpool.tile([128, CCH, 512], F32R)
                nc.sync.dma_start(
                    xs[:, :, :],
                    xT_d[:, 512 * s:512 * (s + 1)].rearrange("(c p) t -> p c t", p=128))
                if s == 0:
                    # batched weight loads, emitted after the first x strip so
                    # the PE can start as soon as wq lands (wq first: q runs first)
                    for w_sb, w_d in ((wq, wq_d), (wk, wk_d), (wv, wv_d)):
                        nc.sync.dma_start(w_sb[:, :, :],
                                          w_d[:, :].rearrange("(c p) o -> p c o", p=128))
                for w_sb, dst in ((wq, qT), (wk, kT)):
                    for o in range(HPC // 2):
                        pq = ps1.tile([128, 512], F32, tag="pp")
                        for c in range(CCH):
                            nc.tensor.matmul(pq[:, :], w_sb[:, c, 128 * o:128 * (o + 1)],
                                             xs[:, c, :], start=(c == 0), stop=(c == CCH - 1))
                        tq = tmp1.tile([128, 512], F32, tag="t1")
                        nc.vector.tensor_copy(tq[:, :], pq[:, :])
                        nc.sync.dma_start(dst[:, o, 512 * s:512 * (s + 1)],
                                          tq[:, :].bitcast(F32R))
                for tt in range(4):
                    pv = ps1.tile([128, 512], F32, tag="pp")
                    for c in range(CCH):
                        nc.tensor.matmul(pv[:, :], xs[:, c, 128 * tt:128 * (tt + 1)],
                                         wv[:, c, :], start=(c == 0), stop=(c == CCH - 1))
                    tv = tmp1.tile([128, 512], F32, tag="t1")
                    nc.vector.tensor_copy(tv[:, :], pv[:, :])
                    nc.sync.dma_start(
                        v[:, 4 * s + tt, :, 0:D],
                        tv[:, :].rearrange("p (h d) -> p h d", h=HPC).bitcast(F32R))

        # ---------------- Phase 2+3: attention + output projection ----------------
        avT = stack.enter_context(tc.tile_pool(name="avt", bufs=1)).tile([128, HPC // 2, T], F32R)
        wp = stack.enter_context(tc.tile_pool(name="wp", bufs=1)).tile([128, GC // 128, C], F32R)
        nc.sync.dma_start(wp[:, :, :], wp_d[:, :].rearrange("(c p) o -> p c o", p=128))

        with tc.tile_pool(name="pt", bufs=8) as pt_pool, \
             tc.tile_pool(name="dd", bufs=4) as d_pool, \
             tc.tile_pool(name="rr", bufs=3) as r_pool, \
             tc.tile_pool(name="avtmp", bufs=3) as avtmp_pool, \
             tc.tile_pool(name="ob", bufs=4) as out_pool, \
             tc.tile_pool(name="ps_s", bufs=4, space="PSUM") as ps_s, \
             tc.tile_pool(name="ps_av", bufs=2, space="PSUM") as ps_av, \
             tc.tile_pool(name="ps_bc", bufs=1, space="PSUM") as ps_bc, \
             tc.tile_pool(name="ps_o", bufs=1, space="PSUM") as ps_o:

            for j in range(NJ):
                for hp in range(HPC // 2):
                    nkc = 4 * (j + 1)
                    # both heads of the pair run interleaved: their scores
                    # matmuls sit in adjacent PE slots with disjoint row
                    # groups (K=64 at partition 0 vs 64) and overlap on HW
                    av0 = ps_av.tile([65, 512], F32, tag="av")
                    av1 = ps_av.tile([65, 512], F32, tag="av")
                    avs = [av0, av1]
                    def emit_scores_exp(i):
                        out = []
                        for par in range(2):
                            pb = 64 * par
                            sps = ps_s.tile([128, 512], F32, tag="s")
                            nc.tensor.matmul(
                                sps[:, :],
                                kT[pb:pb + 64, hp, 128 * i:128 * (i + 1)],
                                qT[pb:pb + 64, hp, 512 * j:512 * (j + 1)],
                                start=True, stop=True)
                            ptile = pt_pool.tile([128, 512], F32R, tag="pt")
                            roff = 128 * i - 512 * j
                            if roff >= 0:
                                # diagonal tile: add -1e9 above the diagonal in
                                # PSUM, then exp only the columns [roff:512] the
                                # AV matmul will consume (cols [0:roff] are
                                # fully masked and skipped outright)
                                nc.vector.tensor_tensor(
                                    sps[:, roff:roff + 128], sps[:, roff:roff + 128],
                                    maskneg[:, :], op=mybir.AluOpType.add)
                                nc.scalar.activation(ptile[:, roff:512], sps[:, roff:512],
                                                     AF.Exp, scale=0.125)
                            else:
                                roff = 0
                                nc.scalar.activation(ptile[:, :], sps[:, :], AF.Exp, scale=0.125)
                            out.append((ptile, roff))
                        return out

                    def emit_av(i, pts):
                        for par in range(2):
                            ptile, roff = pts[par]
                            nc.tensor.matmul(avs[par][:, roff:512], v[:, i, 2 * hp + par, :],
                                             ptile[:, roff:512],
                                             start=(i == 0), stop=(i == nkc - 1))

                    # one-chunk software pipeline: chunk i+1's scores sit ahead
                    # of chunk i's AV matmuls in the PE stream, so AV never
                    # waits out the exp latency
                    prev = emit_scores_exp(0)
                    for i in range(1, nkc):
                        cur = emit_scores_exp(i)
                        emit_av(i - 1, prev)
                        prev = cur
                    emit_av(nkc - 1, prev)
                    for par in range(2):
                        av = avs[par]
                        # single DVE copy frees the AV PSUM bank immediately so
                        # the next head pair's AV matmuls are not gated on the
                        # whole normalize chain
                        avr = avtmp_pool.tile([65, 512], F32, tag="avr")
                        nc.vector.tensor_copy(avr[:, :], av[:, :])
                        # softmax denominators: r = exp(-ln(denom)), broadcast via K=1 matmul
                        dt_ = d_pool.tile([65, 512], F32R, tag="d")
                        nc.scalar.activation(dt_[64:65, :], avr[64:65, :], AF.Ln)
                        nc.scalar.activation(dt_[64:65, :], dt_[64:65, :], AF.Exp, scale=-1.0)
                        bc = ps_bc.tile([128, 512], F32, tag="bc")
                        nc.tensor.matmul(bc[:, :], ones[64:65, :], dt_[64:65, :],
                                         start=True, stop=True)
                        rb = r_pool.tile([64, 512], F32, tag="r")
                        nc.vector.tensor_copy(rb[:, :], bc[0:64, :])
                        avf = avtmp_pool.tile([64, 512], F32, tag="avf")
                        nc.vector.tensor_tensor(avf[:, :], avr[0:64, :], rb[:, :], op=MULT)
                        # DMA moves lanes 0:64 to the destination partitions
                        nc.sync.dma_start(avT[64 * par:64 * par + 64, hp, 512 * j:512 * (j + 1)],
                                          avf[:, :].bitcast(F32R))

                # output projection for the t-tiles whose avT columns just completed
                for tt in range(4 * j, 4 * (j + 1)):
                    ob = out_pool.tile([128, C], F32, tag="ob")
                    for o2 in range(2):
                        po = ps_o.tile([128, 512], F32, tag="o")
                        for c4 in range(GC // 128):
                            nc.tensor.matmul(po[:, :], avT[:, c4, 128 * tt:128 * (tt + 1)],
                                             wp[:, c4, 512 * o2:512 * (o2 + 1)],
                                             start=(c4 == 0), stop=(c4 == GC // 128 - 1))
                        nc.vector.tensor_copy(ob[:, 512 * o2:512 * (o2 + 1)], po[:, :])
                    nc.sync.dma_start(out_d[128 * tt:128 * (tt + 1), :], ob[:, :])
    _split_excess_waits(nc)
    return nc


def _get_program():
    global _PROGRAM
    if _PROGRAM is None:
        _PROGRAM = _build_program()
    return _PROGRAM


def _make_in_maps(x, Wk, Wq, Wv, Wp):
    x = np.asarray(x, dtype=np.float32)
    Wk = np.asarray(Wk, dtype=np.float32)
    Wq = np.asarray(Wq, dtype=np.float32)
    Wv = np.asarray(Wv, dtype=np.float32)
    Wp = np.asarray(Wp, dtype=np.float32)
    in_maps = []
    for core in range(8):
        b, g = core // GROUPS, core % GROUPS
        rows = slice(GC * g, GC * (g + 1))
        in_maps.append({
            "xT": np.ascontiguousarray(x[b].T),                 # [C, T]
            "wqT": np.ascontiguousarray(Wq[rows, :].T),         # [C, GC]
            "wkT": np.ascontiguousarray(Wk[rows, :].T),
            "wvT": np.ascontiguousarray(Wv[rows, :].T),
            "wpT": np.ascontiguousarray(Wp[:, rows].T),         # [GC, C]
        })
    return in_maps


def run(x, Wk, Wq, Wv, Wp, bp, trace=False, **spmd_kwargs):
    nc = _get_program()
    in_maps = _make_in_maps(x, Wk, Wq, Wv, Wp)
    res = run_bass_kernel_spmd(nc, in_maps, list(range(8)), trace=trace, **spmd_kwargs)
    bp = np.asarray(bp, dtype=np.float32)
    out = np.empty((B, T, C), dtype=np.float32)
    for b in range(B):
        out[b] = res.results[GROUPS * b]["outp"] + res.results[GROUPS * b + 1]["outp"] + bp
    return out, res


def kernel(x, Wk, Wq, Wv, Wp, bp):
    out, _ = run(x, Wk, Wq, Wv, Wp, bp)
    return out



# revision 7
# speedup vs baseline: 1.0990x; 1.0990x over previous
"""Multi-head causal self-attention (B=4, T=2048, C=1024, 16 heads) on 8 trn2 cores.

Sharding: data-parallel over batch (4) x tensor-parallel over heads (2 groups of 8).
Core m handles batch m//2, head group m%2. Host pre-transposes x and the weights so
every on-device matmul consumes operands in natural layout (zero on-device
transposes); the output projection partial sums are pair-reduced on host (+bias).

Per-core pipeline (all matmuls fp32r = FP22 multiply, fp32 PSUM accumulate):
  qT[o,t] = Wq_g @ x^T        (lhsT = Wq_g^T chunks, rhs = x^T chunks)
  kT[o,t] likewise; v[t,o]    (lhsT = x^T chunks, rhs = Wv_g^T)
  scores^T[k,q] per head      (lhsT = kT tile [64,128], rhs = qT tile [64,512])
  p = exp(0.125*scores^T)     (ACT, causal mask via memset + triangle multiply)
  [AV^T | denom] = [v|1]^T @ p (ones column of v gives softmax denominators)
  avT = AV^T * exp(-ln(denom)) broadcast via K=1 ones-matmul
  out_partial = avT^T @ Wp_g^T
"""

import numpy as np

import concourse.bass as bass
import concourse.mybir as mybir
import concourse.tile as tile
from concourse.bass_utils import run_bass_kernel_spmd

F32 = mybir.dt.float32
F32R = mybir.dt.float32r
AF = mybir.ActivationFunctionType
MULT = mybir.AluOpType.mult

B, T, C = 4, 2048, 1024
HEADS, D = 16, 64
GROUPS = 2                  # head groups (tensor parallel)
HPC = HEADS // GROUPS       # heads per core = 8
GC = HPC * D                # group channel width = 512
NKC = T // 128              # Tk chunks = 16
NJ = T // 512               # Tq tiles = 4
CCH = C // 128              # contraction chunks = 8
NSTRIP = T // 512           # phase-1 t strips = 4

_PROGRAM = None


def _patch_drain_chunking():
    """The axon walrus build rejects instructions with >~4 sem waits; Tile's
    kernel-tail drain waits on every live semaphore at once. Split it into a
    chain of drains with <=2 waits each."""
    from bass_rust import VectorClock, ScopedClock

    if getattr(tile.TileContext, "_drain_chunk_patched", False):
        return

    def _drain_and_barrier(self, tick_clock, wait_clock):
        gc_vec = list(tick_clock.global_clock)
        nz = [i for i, t in enumerate(gc_vec) if t > 0]
        CHUNK = 1
        for k in range(0, len(nz), CHUNK):
            keep = set(nz[k:k + CHUNK])
            partial = [gc_vec[i] if i in keep else 0 for i in range(len(gc_vec))]
            d = self.nc.sync.drain()
            wait_clock.add_sem_waits(d.ins, ScopedClock({None: VectorClock(partial)}))
        self.nc.all_engine_barrier()
        assert self.sems is not None
        popped = self.nc._tile_sem_poison_stack.pop()
        assert popped is self._sem_poison
        self.nc.clear_and_free_semaphores(list(self.sems.allocated().values()))
        self.nc.all_engine_barrier()

    tile.TileContext._drain_and_barrier = _drain_and_barrier
    tile.TileContext._drain_chunk_patched = True


def _split_excess_waits(nc, maxw=1, maxw_other=None):
    """Walrus rejects instructions carrying more than ~1 sem wait (proven for
    PE matmul S3_LW and the SP drain at 5). Move excess waits onto same-engine
    NoOps inserted immediately before the instruction (engine streams execute
    in bb order, so semantics are preserved). maxw_other, if set, applies to
    non-PE engines."""
    from bass_rust import InstNoOp

    ctr = 0
    for f in nc.m.functions:
        for bb in f.blocks:
            new_insts = []
            for inst in bb.instructions:
                si = inst.sync_info
                waits = list(si.on_wait) if si and si.on_wait else []
                lim = maxw
                if maxw_other is not None and str(inst.engine) != 'EngineType.PE':
                    lim = maxw_other
                maxw_eff = lim
                if len(waits) > maxw_eff:
                    head, rest = waits[:-maxw_eff], waits[-maxw_eff:]
                    for k in range(0, len(head), maxw_eff):
                        ctr += 1
                        new_insts.append(InstNoOp(
                            name=f"waitnop_{ctr}",
                            engine=inst.engine,
                            sync_info=mybir.SyncInfo(
                                on_wait=head[k:k + maxw_eff], on_update=[]),
                        ))
                    inst.sync_info = mybir.SyncInfo(on_wait=rest, on_update=si.on_update)
                new_insts.append(inst)
            bb.instructions = new_insts
    return ctr


def _build_program():
    _patch_drain_chunking()
    nc = bass.Bass()

    xT_d = nc.declare_dram_parameter("xT", [C, T], F32R, isOutput=False)
    wq_d = nc.declare_dram_parameter("wqT", [C, GC], F32R, isOutput=False)
    wk_d = nc.declare_dram_parameter("wkT", [C, GC], F32R, isOutput=False)
    wv_d = nc.declare_dram_parameter("wvT", [C, GC], F32R, isOutput=False)
    wp_d = nc.declare_dram_parameter("wpT", [GC, C], F32R, isOutput=False)
    out_d = nc.declare_dram_parameter("outp", [T, C], F32, isOutput=True)

    from contextlib import ExitStack

    with tile.TileContext(nc) as tc, ExitStack() as stack:
        cpool = stack.enter_context(tc.tile_pool(name="const", bufs=1))
        qkv_pool = stack.enter_context(tc.tile_pool(name="qkv", bufs=1))

        # additive causal mask: 0 where q >= k, -1e9 where q < k (exp -> 0),
        # replicated on both par slots so one strided DVE add masks the pair
        maskneg = cpool.tile([128, 2, 128], F32)
        nc.gpsimd.memset(maskneg[:, :, :], 0.0)
        for s in range(2):
            nc.gpsimd.affine_select(
                out=maskneg[:, s, :], in_=maskneg[:, s, :],
                compare_op=mybir.AluOpType.is_ge, fill=-1e9, base=0,
                pattern=[[1, 128]], channel_multiplier=-1,
            )
        # DVE cannot encode f32r, so f32r tiles are written by ACT/DMA only.
        # ones row is bf16 so the K=1 broadcast matmul can take a DVE-written
        # bf16 rhs (DVE cannot write f32r; verifier demands rounded operands)
        BF16 = mybir.dt.bfloat16
        ones = cpool.tile([128, 128], BF16)
        nc.scalar.activation(ones[64:65, :], ones[64:65, :], AF.Copy, scale=0.0, bias=1.0)

        qT = qkv_pool.tile([128, HPC // 2, T], F32R)   # [c, head-pair, t]
        kT = qkv_pool.tile([128, HPC // 2, T], F32R)
        # v padded with a ones column per head: [t-chunk, head, 65]
        v = qkv_pool.tile([128, NKC, HPC, D + 1], F32R)
        nc.scalar.activation(v[:, :, :, D:D + 1], v[:, :, :, D:D + 1],
                             AF.Copy, scale=0.0, bias=1.0)

        # ---------------- Phase 1: QKV projections ----------------
        with tc.tile_pool(name="w1", bufs=1) as wpool, \
             tc.tile_pool(name="xs", bufs=3) as xpool, \
             tc.tile_pool(name="tmp1", bufs=4) as tmp1, \
             tc.tile_pool(name="ps1", bufs=8, space="PSUM") as ps1:
            wq = wpool.tile([128, CCH, GC], F32R)
            wk = wpool.tile([128, CCH, GC], F32R)
            wv = wpool.tile([128, CCH, GC], F32R)

            for s in range(NSTRIP):
                xs = xpool.tile([128, CCH, 512], F32R)
                nc.sync.dma_start(
                    xs[:, :, :],
                    xT_d[:, 512 * s:512 * (s + 1)].rearrange("(c p) t -> p c t", p=128))
                if s == 0:
                    # batched weight loads, emitted after the first x strip so
                    # the PE can start as soon as wq lands (wq first: q runs first)
                    for w_sb, w_d in ((wq, wq_d), (wk, wk_d), (wv, wv_d)):
                        nc.sync.dma_start(w_sb[:, :, :],
                                          w_d[:, :].rearrange("(c p) o -> p c o", p=128))
                for w_sb, dst in ((wq, qT), (wk, kT)):
                    for o in range(HPC // 2):
                        pq = ps1.tile([128, 512], F32, tag="pp")
                        for c in range(CCH):
                            nc.tensor.matmul(pq[:, :], w_sb[:, c, 128 * o:128 * (o + 1)],
                                             xs[:, c, :], start=(c == 0), stop=(c == CCH - 1))
                        tq = tmp1.tile([128, 512], F32, tag="t1")
                        nc.vector.tensor_copy(tq[:, :], pq[:, :])
                        nc.sync.dma_start(dst[:, o, 512 * s:512 * (s + 1)],
                                          tq[:, :].bitcast(F32R))
                for tt in range(4):
                    pv = ps1.tile([128, 512], F32, tag="pp")
                    for c in range(CCH):
                        nc.tensor.matmul(pv[:, :], xs[:, c, 128 * tt:128 * (tt + 1)],
                                         wv[:, c, :], start=(c == 0), stop=(c == CCH - 1))
                    tv = tmp1.tile([128, 512], F32, tag="t1")
                    nc.vector.tensor_copy(tv[:, :], pv[:, :])
                    nc.sync.dma_start(
                        v[:, 4 * s + tt, :, 0:D],
                        tv[:, :].rearrange("p (h d) -> p h d", h=HPC).bitcast(F32R))

        # ---------------- Phase 2+3: attention + output projection ----------------
        avT = stack.enter_context(tc.tile_pool(name="avt", bufs=1)).tile([128, HPC // 2, T], F32R)
        wp = stack.enter_context(tc.tile_pool(name="wp", bufs=1)).tile([128, GC // 128, C], F32R)
        nc.sync.dma_start(wp[:, :, :], wp_d[:, :].rearrange("(c p) o -> p c o", p=128))

        with tc.tile_pool(name="pt", bufs=4) as pt_pool, \
             tc.tile_pool(name="avtmp", bufs=3) as avtmp_pool, \
             tc.tile_pool(name="ob", bufs=4) as out_pool, \
             tc.tile_pool(name="ps_s", bufs=2, space="PSUM") as ps_s, \
             tc.tile_pool(name="ps_av", bufs=2, space="PSUM") as ps_av, \
             tc.tile_pool(name="ps_bc", bufs=1, space="PSUM") as ps_bc, \
             tc.tile_pool(name="ps_o", bufs=1, space="PSUM") as ps_o:

            for j in range(NJ):
                for hp in range(HPC // 2):
                    nkc = 4 * (j + 1)
                    # both heads of the pair run interleaved: their scores
                    # matmuls sit in adjacent PE slots with disjoint row
                    # groups (K=64 at partition 0 vs 64); the pair shares one
                    # 2-bank PSUM tile so a single strided exp covers both
                    av0 = ps_av.tile([65, 512], F32, tag="av")
                    av1 = ps_av.tile([65, 512], F32, tag="av")
                    avs = [av0, av1]
                    def emit_scores_exp(i):
                        sps = ps_s.tile([128, 2, 512], F32, tag="s")
                        roff = 128 * i - 512 * j
                        diag = roff >= 0
                        roff = max(roff, 0)
                        # f32r matmuls need free>=256 for full rate; columns
                        # below the consumed range are computed-but-unused
                        moff = min(roff, 256)
                        for par in range(2):
                            pb = 64 * par
                            nc.tensor.matmul(
                                sps[:, par, moff:512],
                                kT[pb:pb + 64, hp, 128 * i:128 * (i + 1)],
                                qT[pb:pb + 64, hp, 512 * j + moff:512 * (j + 1)],
                                start=True, stop=True)
                        ptile = pt_pool.tile([128, 2, 512], F32R, tag="pt")
                        if diag:
                            # add -1e9 above the diagonal for both pars at once
                            nc.vector.tensor_tensor(
                                sps[:, :, roff:roff + 128], sps[:, :, roff:roff + 128],
                                maskneg[:, :, :], op=mybir.AluOpType.add)
                        # one exp for the par pair; cols [0:roff] are fully
                        # masked and skipped outright
                        nc.scalar.activation(ptile[:, :, roff:512], sps[:, :, roff:512],
                                             AF.Exp, scale=0.125)
                        return ptile, roff

                    def emit_av(i, pts):
                        ptile, roff = pts
                        for par in range(2):
                            nc.tensor.matmul(avs[par][:, roff:512], v[:, i, 2 * hp + par, :],
                                             ptile[:, par, roff:512],
                                             start=(i == 0), stop=(i == nkc - 1))

                    # one-chunk software pipeline: chunk i+1's scores sit ahead
                    # of chunk i's AV matmuls in the PE stream, so AV never
                    # waits out the exp latency
                    prev = emit_scores_exp(0)
                    for i in range(1, nkc):
                        cur = emit_scores_exp(i)
                        emit_av(i - 1, prev)
                        prev = cur
                    emit_av(nkc - 1, prev)
                    for par in range(2):
                        av = avs[par]
                        # single DVE copy frees the AV PSUM bank immediately so
                        # the next head pair's AV matmuls are not gated on the
                        # whole normalize chain
                        avr = avtmp_pool.tile([65, 512], F32, tag="avr")
                        nc.vector.tensor_copy(avr[:, :], av[:, :])
                        # softmax denominators: bf16 reciprocal row on DVE,
                        # broadcast to 128 partitions via K=1 matmul, multiply
                        rcp = avtmp_pool.tile([65, 512], BF16, tag="rcp")
                        with nc.allow_low_precision(reason="bf16 denom recip: 0.4% on softmax scale"):
                            nc.vector.reciprocal(rcp[64:65, :], avr[64:65, :])
                        bc = ps_bc.tile([128, 512], F32, tag="bc")
                        nc.tensor.matmul(bc[:, :], ones[64:65, :], rcp[64:65, :],
                                         start=True, stop=True)
                        avf = avtmp_pool.tile([64, 512], F32, tag="avf")
                        nc.vector.tensor_tensor(avf[:, :], avr[0:64, :], bc[0:64, :],
                                                op=MULT)
                        # DMA moves lanes 0:64 to the destination partitions
                        nc.sync.dma_start(avT[64 * par:64 * par + 64, hp, 512 * j:512 * (j + 1)],
                                          avf[:, :].bitcast(F32R))

                # output projection for the t-tiles whose avT columns just completed
                for tt in range(4 * j, 4 * (j + 1)):
                    ob = out_pool.tile([128, C], F32, tag="ob")
                    for o2 in range(2):
                        po = ps_o.tile([128, 512], F32, tag="o")
                        for c4 in range(GC // 128):
                            nc.tensor.matmul(po[:, :], avT[:, c4, 128 * tt:128 * (tt + 1)],
                                             wp[:, c4, 512 * o2:512 * (o2 + 1)],
                                             start=(c4 == 0), stop=(c4 == GC // 128 - 1))
                        nc.vector.tensor_copy(ob[:, 512 * o2:512 * (o2 + 1)], po[:, :])
                    nc.sync.dma_start(out_d[128 * tt:128 * (tt + 1), :], ob[:, :])
    _split_excess_waits(nc)
    return nc


def _get_program():
    global _PROGRAM
    if _PROGRAM is None:
        _PROGRAM = _build_program()
    return _PROGRAM


def _make_in_maps(x, Wk, Wq, Wv, Wp):
    x = np.asarray(x, dtype=np.float32)
    Wk = np.asarray(Wk, dtype=np.float32)
    Wq = np.asarray(Wq, dtype=np.float32)
    Wv = np.asarray(Wv, dtype=np.float32)
    Wp = np.asarray(Wp, dtype=np.float32)
    in_maps = []
    for core in range(8):
        b, g = core // GROUPS, core % GROUPS
        rows = slice(GC * g, GC * (g + 1))
        in_maps.append({
            "xT": np.ascontiguousarray(x[b].T),                 # [C, T]
            "wqT": np.ascontiguousarray(Wq[rows, :].T),         # [C, GC]
            "wkT": np.ascontiguousarray(Wk[rows, :].T),
            "wvT": np.ascontiguousarray(Wv[rows, :].T),
            "wpT": np.ascontiguousarray(Wp[:, rows].T),         # [GC, C]
        })
    return in_maps


def run(x, Wk, Wq, Wv, Wp, bp, trace=False, **spmd_kwargs):
    nc = _get_program()
    in_maps = _make_in_maps(x, Wk, Wq, Wv, Wp)
    res = run_bass_kernel_spmd(nc, in_maps, list(range(8)), trace=trace, **spmd_kwargs)
    bp = np.asarray(bp, dtype=np.float32)
    out = np.empty((B, T, C), dtype=np.float32)
    for b in range(B):
        out[b] = res.results[GROUPS * b]["outp"] + res.results[GROUPS * b + 1]["outp"] + bp
    return out, res


def kernel(x, Wk, Wq, Wv, Wp, bp):
    out, _ = run(x, Wk, Wq, Wv, Wp, bp)
    return out



# revision 16
# speedup vs baseline: 1.2244x; 1.1142x over previous
"""Multi-head causal self-attention (B=4, T=2048, C=1024, 16 heads) on 8 trn2 cores.

Sharding: data-parallel over batch (4) x tensor-parallel over heads (2 groups of 8).
Core m handles batch m//2, head group m%2. Host pre-transposes x and the weights so
every on-device matmul consumes operands in natural layout (zero on-device
transposes); the output projection partial sums are pair-reduced on host (+bias).

Per-core pipeline (all matmuls fp32r = FP22 multiply, fp32 PSUM accumulate):
  qT[o,t] = Wq_g @ x^T        (lhsT = Wq_g^T chunks, rhs = x^T chunks)
  kT[o,t] likewise; v[t,o]    (lhsT = x^T chunks, rhs = Wv_g^T)
  scores^T[k,q] per head      (lhsT = kT tile [64,128], rhs = qT tile [64,512])
  p = exp(0.125*scores^T)     (ACT, causal mask via memset + triangle multiply)
  [AV^T | denom] = [v|1]^T @ p (ones column of v gives softmax denominators)
  avT = AV^T * exp(-ln(denom)) broadcast via K=1 ones-matmul
  out_partial = avT^T @ Wp_g^T
"""

import numpy as np

import concourse.bass as bass
import concourse.mybir as mybir
import concourse.tile as tile
from concourse.bass_utils import run_bass_kernel_spmd

F32 = mybir.dt.float32
F32R = mybir.dt.float32r
F8 = mybir.dt.float8e4
BF16 = mybir.dt.bfloat16
DR = mybir.MatmulPerfMode.DoubleRow
AF = mybir.ActivationFunctionType
MULT = mybir.AluOpType.mult
WSCALE = 32.0               # host scales Wq/Wk by 32 so fp8 stays out of subnormals
EXP_SCALE = 0.125 / (WSCALE * WSCALE)

B, T, C = 4, 2048, 1024
HEADS, D = 16, 64
GROUPS = 2                  # head groups (tensor parallel)
HPC = HEADS // GROUPS       # heads per core = 8
GC = HPC * D                # group channel width = 512
NKC = T // 128              # Tk chunks = 16
NJ = T // 512               # Tq tiles = 4
CCH = C // 128              # contraction chunks = 8
NSTRIP = T // 512           # phase-1 t strips = 4

_PROGRAM = None


def _patch_drain_chunking():
    """The axon walrus build rejects instructions with >~4 sem waits; Tile's
    kernel-tail drain waits on every live semaphore at once. Split it into a
    chain of drains with <=2 waits each."""
    from bass_rust import VectorClock, ScopedClock

    if getattr(tile.TileContext, "_drain_chunk_patched", False):
        return

    def _drain_and_barrier(self, tick_clock, wait_clock):
        gc_vec = list(tick_clock.global_clock)
        nz = [i for i, t in enumerate(gc_vec) if t > 0]
        CHUNK = 1
        for k in range(0, len(nz), CHUNK):
            keep = set(nz[k:k + CHUNK])
            partial = [gc_vec[i] if i in keep else 0 for i in range(len(gc_vec))]
            d = self.nc.sync.drain()
            wait_clock.add_sem_waits(d.ins, ScopedClock({None: VectorClock(partial)}))
        self.nc.all_engine_barrier()
        assert self.sems is not None
        popped = self.nc._tile_sem_poison_stack.pop()
        assert popped is self._sem_poison
        self.nc.clear_and_free_semaphores(list(self.sems.allocated().values()))
        self.nc.all_engine_barrier()

    tile.TileContext._drain_and_barrier = _drain_and_barrier
    tile.TileContext._drain_chunk_patched = True


def _split_excess_waits(nc, maxw=1, maxw_other=None):
    """Walrus rejects instructions carrying more than ~1 sem wait (proven for
    PE matmul S3_LW and the SP drain at 5). Move excess waits onto same-engine
    NoOps inserted immediately before the instruction (engine streams execute
    in bb order, so semantics are preserved). maxw_other, if set, applies to
    non-PE engines."""
    from bass_rust import InstNoOp

    ctr = 0
    for f in nc.m.functions:
        for bb in f.blocks:
            new_insts = []
            for inst in bb.instructions:
                si = inst.sync_info
                waits = list(si.on_wait) if si and si.on_wait else []
                lim = maxw
                if maxw_other is not None and str(inst.engine) != 'EngineType.PE':
                    lim = maxw_other
                maxw_eff = lim
                if len(waits) > maxw_eff:
                    head, rest = waits[:-maxw_eff], waits[-maxw_eff:]
                    for k in range(0, len(head), maxw_eff):
                        ctr += 1
                        new_insts.append(InstNoOp(
                            name=f"waitnop_{ctr}",
                            engine=inst.engine,
                            sync_info=mybir.SyncInfo(
                                on_wait=head[k:k + maxw_eff], on_update=[]),
                        ))
                    inst.sync_info = mybir.SyncInfo(on_wait=rest, on_update=si.on_update)
                new_insts.append(inst)
            bb.instructions = new_insts
    return ctr


def _build_program():
    _patch_drain_chunking()
    nc = bass.Bass()

    xT_d = nc.declare_dram_parameter("xT", [C, T], F32R, isOutput=False)
    xT8_d = nc.declare_dram_parameter("xT8", [C, T], F8, isOutput=False)
    wq8_d = nc.declare_dram_parameter("wq8", [C, GC], F8, isOutput=False)
    wk8_d = nc.declare_dram_parameter("wk8", [C, GC], F8, isOutput=False)
    wv_d = nc.declare_dram_parameter("wvT", [C, GC], F32R, isOutput=False)
    wp_d = nc.declare_dram_parameter("wpT", [GC, C], F32R, isOutput=False)
    out_d = nc.declare_dram_parameter("outp", [T, C], F32, isOutput=True)

    from contextlib import ExitStack

    with tile.TileContext(nc) as tc, ExitStack() as stack:
        cpool = stack.enter_context(tc.tile_pool(name="const", bufs=1))
        qkv_pool = stack.enter_context(tc.tile_pool(name="qkv", bufs=1))

        # additive causal mask: 0 where q >= k, -1e9 where q < k (exp -> 0),
        # replicated on both par slots so one strided DVE add masks the pair
        maskneg = cpool.tile([128, 2, 128], F32)
        nc.gpsimd.memset(maskneg[:, :, :], 0.0)
        for s in range(2):
            nc.gpsimd.affine_select(
                out=maskneg[:, s, :], in_=maskneg[:, s, :],
                compare_op=mybir.AluOpType.is_ge, fill=-1e9, base=0,
                pattern=[[1, 128]], channel_multiplier=-1,
            )
        # DVE cannot encode f32r, so f32r tiles are written by ACT/DMA only.
        # ones row is bf16 so the K=1 broadcast matmul can take a DVE-written
        # bf16 rhs (DVE cannot write f32r; verifier demands rounded operands)
        BF16 = mybir.dt.bfloat16
        ones = cpool.tile([128, 128], BF16)
        nc.scalar.activation(ones[64:65, :], ones[64:65, :], AF.Copy, scale=0.0, bias=1.0)

        # fp8 q/k for DoubleRow scores: partition = 32*(h%2) + d%32 (PE operand
        # base partition must be 0/32/64), free = [d//32 (stacked contraction
        # half), h//2, t]
        qT8 = qkv_pool.tile([64, 2, HPC // 2, T], F8)
        kT8 = qkv_pool.tile([64, 2, HPC // 2, T], F8)
        # v padded with a ones column per head: [t-chunk, head, 65]
        v = qkv_pool.tile([128, NKC, HPC, D + 1], F32R)
        nc.scalar.activation(v[:, :, :, D:D + 1], v[:, :, :, D:D + 1],
                             AF.Copy, scale=0.0, bias=1.0)

        # ---------------- Phase 1: QKV projections ----------------
        # Q,K run as fp8 DoubleRow matmuls (host supplies x and 32*W in fp8,
        # weight columns pre-permuted so the fp8 eviction DMAs are pure
        # partition-offset moves); V runs in f32r off the f32r x copy.
        with tc.tile_pool(name="w1", bufs=1) as wpool, \
             tc.tile_pool(name="xs", bufs=2) as xpool, \
             tc.tile_pool(name="xs8", bufs=2) as x8pool, \
             tc.tile_pool(name="stg", bufs=2) as stgpool, \
             tc.tile_pool(name="tmp1", bufs=4) as tmp1, \
             tc.tile_pool(name="ps1", bufs=8, space="PSUM") as ps1:
            wq8 = wpool.tile([128, CCH, GC], F8)
            wk8 = wpool.tile([128, CCH, GC], F8)
            wv = wpool.tile([128, CCH, GC], F32R)

            for s in range(NSTRIP):
                xs8 = x8pool.tile([128, CCH, 512], F8)
                nc.sync.dma_start(
                    xs8[:, :, :],
                    xT8_d[:, 512 * s:512 * (s + 1)].rearrange("(c p) t -> p c t", p=128))
                if s == 0:
                    # small fp8 weight loads right after the (small) fp8 x strip
                    # so the first Q DoubleRow matmul starts ~4us in; the big
                    # f32r x strip and wv follow (V matmuls run last anyway)
                    for w_sb, w_d in ((wq8, wq8_d), (wk8, wk8_d)):
                        nc.sync.dma_start(w_sb[:, :, :],
                                          w_d[:, :].rearrange("(c p) o -> p c o", p=128))
                xs = xpool.tile([128, CCH, 512], F32R)
                nc.sync.dma_start(
                    xs[:, :, :],
                    xT_d[:, 512 * s:512 * (s + 1)].rearrange("(c p) t -> p c t", p=128))
                if s == 0:
                    nc.sync.dma_start(wv[:, :, :],
                                      wv_d[:, :].rearrange("(c p) o -> p c o", p=128))
                for w_sb, dst, stag in ((wq8, qT8, "sq"), (wk8, kT8, "sk")):
                    stage = stgpool.tile([128, HPC // 2, 512], F8, tag=stag)
                    for o in range(HPC // 2):
                        pq = ps1.tile([128, 512], F32, tag="pp")
                        for c2 in range(CCH // 2):
                            nc.tensor.matmul(pq[:, :],
                                             w_sb[:, 2 * c2:2 * c2 + 2, 128 * o:128 * (o + 1)],
                                             xs8[:, 2 * c2:2 * c2 + 2, :],
                                             start=(c2 == 0), stop=(c2 == CCH // 2 - 1),
                                             perf_mode=DR)
                        # fp8 eviction on the otherwise-idle ACT engine
                        nc.scalar.activation(stage[:, o, :], pq[:, :], AF.Copy)
                    # staging row 64*sh+32*hh+d32 at slot o holds head 2o+hh,
                    # d=32*sh+d32 (host permuted W cols); move to partition
                    # 32*hh+d32, slot [sh, o]
                    for sh in range(2):
                        nc.sync.dma_start(
                            dst[:, sh, :, 512 * s:512 * (s + 1)],
                            stage[64 * sh:64 * sh + 64, :, :])
                for tt in range(4):
                    pv = ps1.tile([128, 512], F32, tag="pp")
                    for c in range(CCH):
                        nc.tensor.matmul(pv[:, :], xs[:, c, 128 * tt:128 * (tt + 1)],
                                         wv[:, c, :], start=(c == 0), stop=(c == CCH - 1))
                    tv = tmp1.tile([128, 512], F32, tag="t1")
                    nc.vector.tensor_copy(tv[:, :], pv[:, :])
                    nc.sync.dma_start(
                        v[:, 4 * s + tt, :, 0:D],
                        tv[:, :].rearrange("p (h d) -> p h d", h=HPC).bitcast(F32R))

        # ---------------- Phase 2+3: attention + output projection ----------------
        avT = stack.enter_context(tc.tile_pool(name="avt", bufs=1)).tile([128, HPC // 2, T], F32R)
        wp = stack.enter_context(tc.tile_pool(name="wp", bufs=1)).tile([128, GC // 128, C], F32R)
        nc.sync.dma_start(wp[:, :, :], wp_d[:, :].rearrange("(c p) o -> p c o", p=128))

        with tc.tile_pool(name="pt", bufs=4) as pt_pool, \
             tc.tile_pool(name="avtmp", bufs=3) as avtmp_pool, \
             tc.tile_pool(name="ob", bufs=4) as out_pool, \
             tc.tile_pool(name="ps_s", bufs=2, space="PSUM") as ps_s, \
             tc.tile_pool(name="ps_av", bufs=2, space="PSUM") as ps_av, \
             tc.tile_pool(name="ps_bc", bufs=1, space="PSUM") as ps_bc, \
             tc.tile_pool(name="ps_o", bufs=1, space="PSUM") as ps_o:

            for j in range(NJ):
                for hp in range(HPC // 2):
                    nkc = 4 * (j + 1)
                    # both heads of the pair run interleaved: their scores
                    # matmuls sit in adjacent PE slots with disjoint row
                    # groups (K=64 at partition 0 vs 64); the pair shares one
                    # 2-bank PSUM tile so a single strided exp covers both
                    av0 = ps_av.tile([65, 512], F32, tag="av")
                    av1 = ps_av.tile([65, 512], F32, tag="av")
                    avs = [av0, av1]
                    def emit_scores_exp(i):
                        sps = ps_s.tile([128, 2, 512], F32, tag="s")
                        roff = 128 * i - 512 * j
                        diag = roff >= 0
                        roff = max(roff, 0)
                        for par in range(2):
                            h = 2 * hp + par
                            hb, hq = h % 2, h // 2
                            nc.tensor.matmul(
                                sps[:, par, roff:512],
                                kT8[32 * hb:32 * hb + 32, :, hq, 128 * i:128 * (i + 1)],
                                qT8[32 * hb:32 * hb + 32, :, hq, 512 * j + roff:512 * (j + 1)],
                                start=True, stop=True, perf_mode=DR)
                        ptile = pt_pool.tile([128, 2, 512], F32R, tag="pt")
                        if diag:
                            # add -1e9 above the diagonal for both pars at once
                            nc.vector.tensor_tensor(
                                sps[:, :, roff:roff + 128], sps[:, :, roff:roff + 128],
                                maskneg[:, :, :], op=mybir.AluOpType.add)
                        # one exp for the par pair; cols [0:roff] are fully
                        # masked and skipped outright
                        nc.scalar.activation(ptile[:, :, roff:512], sps[:, :, roff:512],
                                             AF.Exp, scale=EXP_SCALE)
                        return ptile, roff

                    def emit_av(i, pts):
                        ptile, roff = pts
                        for par in range(2):
                            nc.tensor.matmul(avs[par][:, roff:512], v[:, i, 2 * hp + par, :],
                                             ptile[:, par, roff:512],
                                             start=(i == 0), stop=(i == nkc - 1))

                    # one-chunk software pipeline: chunk i+1's scores sit ahead
                    # of chunk i's AV matmuls in the PE stream, so AV never
                    # waits out the exp latency
                    prev = emit_scores_exp(0)
                    for i in range(1, nkc):
                        cur = emit_scores_exp(i)
                        emit_av(i - 1, prev)
                        prev = cur
                    emit_av(nkc - 1, prev)
                    for par in range(2):
                        av = avs[par]
                        # single DVE copy frees the AV PSUM bank immediately so
                        # the next head pair's AV matmuls are not gated on the
                        # whole normalize chain
                        avr = avtmp_pool.tile([65, 512], F32, tag="avr")
                        nc.vector.tensor_copy(avr[:, :], av[:, :])
                        # softmax denominators: bf16 reciprocal row on DVE,
                        # broadcast to 128 partitions via K=1 matmul, multiply
                        rcp = avtmp_pool.tile([65, 512], BF16, tag="rcp")
                        with nc.allow_low_precision(reason="bf16 denom recip: 0.4% on softmax scale"):
                            nc.vector.reciprocal(rcp[64:65, :], avr[64:65, :])
                        bc = ps_bc.tile([128, 512], F32, tag="bc")
                        nc.tensor.matmul(bc[:, :], ones[64:65, :], rcp[64:65, :],
                                         start=True, stop=True)
                        avf = avtmp_pool.tile([64, 512], F32, tag="avf")
                        nc.vector.tensor_tensor(avf[:, :], avr[0:64, :], bc[0:64, :],
                                                op=MULT)
                        # DMA moves lanes 0:64 to the destination partitions
                        nc.sync.dma_start(avT[64 * par:64 * par + 64, hp, 512 * j:512 * (j + 1)],
                                          avf[:, :].bitcast(F32R))

                # output projection for the t-tiles whose avT columns just completed
                for tt in range(4 * j, 4 * (j + 1)):
                    ob = out_pool.tile([128, C], F32, tag="ob")
                    for o2 in range(2):
                        po = ps_o.tile([128, 512], F32, tag="o")
                        for c4 in range(GC // 128):
                            nc.tensor.matmul(po[:, :], avT[:, c4, 128 * tt:128 * (tt + 1)],
                                             wp[:, c4, 512 * o2:512 * (o2 + 1)],
                                             start=(c4 == 0), stop=(c4 == GC // 128 - 1))
                        nc.vector.tensor_copy(ob[:, 512 * o2:512 * (o2 + 1)], po[:, :])
                    nc.sync.dma_start(out_d[128 * tt:128 * (tt + 1), :], ob[:, :])
    _split_excess_waits(nc)
    return nc


def _get_program():
    global _PROGRAM
    if _PROGRAM is None:
        _PROGRAM = _build_program()
    return _PROGRAM


def _make_in_maps(x, Wk, Wq, Wv, Wp):
    import ml_dtypes
    f8 = ml_dtypes.float8_e4m3
    x = np.asarray(x, dtype=np.float32)
    Wk = np.asarray(Wk, dtype=np.float32)
    Wq = np.asarray(Wq, dtype=np.float32)
    Wv = np.asarray(Wv, dtype=np.float32)
    Wp = np.asarray(Wp, dtype=np.float32)
    # fp8 weight column permutation: col 128*o+64*sh+32*hh+d32 holds original
    # out-channel (2*o+hh)*64 + 32*sh + d32, so the q/k eviction DMA is a pure
    # partition-offset move into the [32*(h%4)+d32, d//32, h//4, t] layout
    perm = np.empty(GC, np.int64)
    for o in range(4):
        for sh in range(2):
            for hh in range(2):
                base = 128 * o + 64 * sh + 32 * hh
                perm[base:base + 32] = (2 * o + hh) * 64 + 32 * sh + np.arange(32)
    in_maps = []
    for core in range(8):
        b, g = core // GROUPS, core % GROUPS
        rows = slice(GC * g, GC * (g + 1))
        xT = np.ascontiguousarray(x[b].T)
        in_maps.append({
            "xT": xT,                                           # [C, T]
            "xT8": np.ascontiguousarray(xT.astype(f8)),
            "wq8": np.ascontiguousarray((WSCALE * Wq[rows, :][perm, :].T).astype(f8)),
            "wk8": np.ascontiguousarray((WSCALE * Wk[rows, :][perm, :].T).astype(f8)),
            "wvT": np.ascontiguousarray(Wv[rows, :].T),
            "wpT": np.ascontiguousarray(Wp[:, rows].T),         # [GC, C]
        })
    return in_maps


def run(x, Wk, Wq, Wv, Wp, bp, trace=False, **spmd_kwargs):
    nc = _get_program()
    in_maps = _make_in_maps(x, Wk, Wq, Wv, Wp)
    res = run_bass_kernel_spmd(nc, in_maps, list(range(8)), trace=trace, **spmd_kwargs)
    bp = np.asarray(bp, dtype=np.float32)
    out = np.empty((B, T, C), dtype=np.float32)
    for b in range(B):
        out[b] = res.results[GROUPS * b]["outp"] + res.results[GROUPS * b + 1]["outp"] + bp
    return out, res


def kernel(x, Wk, Wq, Wv, Wp, bp):
    out, _ = run(x, Wk, Wq, Wv, Wp, bp)
    return out



# revision 33
# speedup vs baseline: 1.2957x; 1.0583x over previous
"""Multi-head causal self-attention (B=4, T=2048, C=1024, 16 heads) on 8 trn2 cores.

Sharding: data-parallel over batch (4) x tensor-parallel over heads (2 groups of 8).
Core m handles batch m//2, head group m%2. Host pre-transposes x and the weights so
every on-device matmul consumes operands in natural layout (zero on-device
transposes); the output projection partial sums are pair-reduced on host (+bias).

Per-core pipeline (all matmuls fp32r = FP22 multiply, fp32 PSUM accumulate):
  qT[o,t] = Wq_g @ x^T        (lhsT = Wq_g^T chunks, rhs = x^T chunks)
  kT[o,t] likewise; v[t,o]    (lhsT = x^T chunks, rhs = Wv_g^T)
  scores^T[k,q] per head      (lhsT = kT tile [64,128], rhs = qT tile [64,512])
  p = exp(0.125*scores^T)     (ACT, causal mask via memset + triangle multiply)
  [AV^T | denom] = [v|1]^T @ p (ones column of v gives softmax denominators)
  avT = AV^T * exp(-ln(denom)) broadcast via K=1 ones-matmul
  out_partial = avT^T @ Wp_g^T
"""

import numpy as np

import concourse.bass as bass
import concourse.mybir as mybir
import concourse.tile as tile
from concourse.bass_utils import run_bass_kernel_spmd

F32 = mybir.dt.float32
F32R = mybir.dt.float32r
F8 = mybir.dt.float8e4
BF16 = mybir.dt.bfloat16
DR = mybir.MatmulPerfMode.DoubleRow
AF = mybir.ActivationFunctionType
MULT = mybir.AluOpType.mult
WSCALE = 32.0               # host scales Wq/Wk by 32 so fp8 stays out of subnormals
EXP_SCALE = 0.125 / (WSCALE * WSCALE)

B, T, C = 4, 2048, 1024
HEADS, D = 16, 64
GROUPS = 2                  # head groups (tensor parallel)
HPC = HEADS // GROUPS       # heads per core = 8
GC = HPC * D                # group channel width = 512
NKC = T // 128              # Tk chunks = 16
NJ = T // 512               # Tq tiles = 4
CCH = C // 128              # contraction chunks = 8
NSTRIP = T // 512           # phase-1 t strips = 4

_PROGRAM = None


def _patch_drain_chunking():
    """The axon walrus build rejects instructions with >~4 sem waits; Tile's
    kernel-tail drain waits on every live semaphore at once. Split it into a
    chain of drains with <=2 waits each."""
    from bass_rust import VectorClock, ScopedClock

    if getattr(tile.TileContext, "_drain_chunk_patched", False):
        return

    def _drain_and_barrier(self, tick_clock, wait_clock):
        gc_vec = list(tick_clock.global_clock)
        nz = [i for i, t in enumerate(gc_vec) if t > 0]
        CHUNK = 1
        for k in range(0, len(nz), CHUNK):
            keep = set(nz[k:k + CHUNK])
            partial = [gc_vec[i] if i in keep else 0 for i in range(len(gc_vec))]
            d = self.nc.sync.drain()
            wait_clock.add_sem_waits(d.ins, ScopedClock({None: VectorClock(partial)}))
        self.nc.all_engine_barrier()
        assert self.sems is not None
        popped = self.nc._tile_sem_poison_stack.pop()
        assert popped is self._sem_poison
        self.nc.clear_and_free_semaphores(list(self.sems.allocated().values()))
        self.nc.all_engine_barrier()

    tile.TileContext._drain_and_barrier = _drain_and_barrier
    tile.TileContext._drain_chunk_patched = True


def _split_excess_waits(nc, maxw=1, maxw_other=None):
    """Walrus rejects instructions carrying more than ~1 sem wait (proven for
    PE matmul S3_LW and the SP drain at 5). Move excess waits onto same-engine
    NoOps inserted immediately before the instruction (engine streams execute
    in bb order, so semantics are preserved). maxw_other, if set, applies to
    non-PE engines."""
    from bass_rust import InstNoOp

    ctr = 0
    for f in nc.m.functions:
        for bb in f.blocks:
            new_insts = []
            for inst in bb.instructions:
                si = inst.sync_info
                waits = list(si.on_wait) if si and si.on_wait else []
                lim = maxw
                if maxw_other is not None and str(inst.engine) != 'EngineType.PE':
                    lim = maxw_other
                maxw_eff = lim
                if len(waits) > maxw_eff:
                    head, rest = waits[:-maxw_eff], waits[-maxw_eff:]
                    for k in range(0, len(head), maxw_eff):
                        ctr += 1
                        new_insts.append(InstNoOp(
                            name=f"waitnop_{ctr}",
                            engine=inst.engine,
                            sync_info=mybir.SyncInfo(
                                on_wait=head[k:k + maxw_eff], on_update=[]),
                        ))
                    inst.sync_info = mybir.SyncInfo(on_wait=rest, on_update=si.on_update)
                new_insts.append(inst)
            bb.instructions = new_insts
    return ctr


def _build_program():
    _patch_drain_chunking()
    nc = bass.Bass()

    xT_d = nc.declare_dram_parameter("xT", [C, T], F32R, isOutput=False)
    xT8_d = nc.declare_dram_parameter("xT8", [C, T], F8, isOutput=False)
    wq8_d = nc.declare_dram_parameter("wq8", [C, GC], F8, isOutput=False)
    wk8_d = nc.declare_dram_parameter("wk8", [C, GC], F8, isOutput=False)
    wv_d = nc.declare_dram_parameter("wvT", [C, GC], F32R, isOutput=False)
    wp_d = nc.declare_dram_parameter("wpT", [GC, C], F32R, isOutput=False)
    out_d = nc.declare_dram_parameter("outp", [T, C], F32, isOutput=True)

    from contextlib import ExitStack

    with tile.TileContext(nc) as tc, ExitStack() as stack:
        cpool = stack.enter_context(tc.tile_pool(name="const", bufs=1))
        qkv_pool = stack.enter_context(tc.tile_pool(name="qkv", bufs=1))

        # additive causal mask: 0 where q >= k, -1e9 where q < k (exp -> 0),
        # replicated on both par slots so one strided DVE add masks the pair
        maskneg = cpool.tile([128, 2, 128], F32)
        nc.gpsimd.memset(maskneg[:, :, :], 0.0)
        for s in range(2):
            nc.gpsimd.affine_select(
                out=maskneg[:, s, :], in_=maskneg[:, s, :],
                compare_op=mybir.AluOpType.is_ge, fill=-1e9, base=0,
                pattern=[[1, 128]], channel_multiplier=-1,
            )
        # DVE cannot encode f32r, so f32r tiles are written by ACT/DMA only.
        # ones row is bf16 so the K=1 broadcast matmul can take a DVE-written
        # bf16 rhs (DVE cannot write f32r; verifier demands rounded operands)
        BF16 = mybir.dt.bfloat16
        ones = cpool.tile([128, 128], BF16)
        nc.scalar.activation(ones[64:65, :], ones[64:65, :], AF.Copy, scale=0.0, bias=1.0)

        # fp8 q/k for DoubleRow scores: partition = 32*(h%2) + d%32 (PE operand
        # base partition must be 0/32/64), free = [d//32 (stacked contraction
        # half), h//2, t]
        qT8 = qkv_pool.tile([64, 2, HPC // 2, T], F8)
        kT8 = qkv_pool.tile([64, 2, HPC // 2, T], F8)
        # v padded with a ones row per head: [t-chunk, d, head] (d-major via
        # host wv column permutation so the eviction DMA moves 2KB-contiguous
        # runs); row 64 = ones for the softmax denominators
        v = qkv_pool.tile([128, NKC, D + 1, HPC], F32R)
        nc.scalar.activation(v[:, :, D, :], v[:, :, D, :],
                             AF.Copy, scale=0.0, bias=1.0)

        avT = qkv_pool.tile([128, HPC // 2, T], F32R)
        wp = qkv_pool.tile([128, GC // 128, C], F32R)

        # Interleaved schedule: phase-1 strip s immediately followed by the
        # attention j-block j=s (its queries live in strip s; its keys/values
        # in strips 0..s), so the SP-bound projection DMAs and the ACT-bound
        # exp stream overlap instead of running as separate phases.
        # PSUM budget (8 banks): ps_s 2x2 + ps_av 2 + ps_x 2 (shared by the
        # qkv projection accumulators, the denom broadcast, and out-proj).
        with tc.tile_pool(name="w1", bufs=1) as wpool, \
             tc.tile_pool(name="xs", bufs=2) as xpool, \
             tc.tile_pool(name="xs8", bufs=1) as x8pool, \
             tc.tile_pool(name="stg", bufs=1) as stgpool, \
             tc.tile_pool(name="tmp1", bufs=2) as tmp1, \
             tc.tile_pool(name="pt", bufs=2) as pt_pool, \
             tc.tile_pool(name="avtmp", bufs=2) as avtmp_pool, \
             tc.tile_pool(name="ob", bufs=2) as out_pool, \
             tc.tile_pool(name="ps_s", bufs=2, space="PSUM") as ps_s, \
             tc.tile_pool(name="ps_av", bufs=2, space="PSUM") as ps_av, \
             tc.tile_pool(name="ps_x", bufs=2, space="PSUM") as ps_x:
            wq8 = wpool.tile([128, CCH, GC], F8)
            wk8 = wpool.tile([128, CCH, GC], F8)
            wv = wpool.tile([128, CCH, GC], F32R)

            def emit_strip(s):
                xs8 = x8pool.tile([128, CCH, 512], F8)
                nc.sync.dma_start(
                    xs8[:, :, :],
                    xT8_d[:, 512 * s:512 * (s + 1)].rearrange("(c p) t -> p c t", p=128))
                if s == 0:
                    # small fp8 weight loads right after the (small) fp8 x strip
                    # so the first Q DoubleRow matmul starts ~4us in; the big
                    # f32r wv/x-strip transfers follow (V matmuls run last)
                    for w_sb, w_d in ((wq8, wq8_d), (wk8, wk8_d)):
                        nc.sync.dma_start(w_sb[:, :, :],
                                          w_d[:, :].rearrange("(c p) o -> p c o", p=128))
                    nc.sync.dma_start(wv[:, :, :],
                                      wv_d[:, :].rearrange("(c p) o -> p c o", p=128))
                # split the f32r strip so V's first contraction chunks can
                # start before the whole strip lands
                xs = xpool.tile([128, CCH, 512], F32R)
                for ch in range(2):
                    cs = slice(4 * ch, 4 * (ch + 1))
                    nc.sync.dma_start(
                        xs[:, cs, :],
                        xT_d[128 * cs.start:128 * cs.stop,
                             512 * s:512 * (s + 1)].rearrange("(c p) t -> p c t", p=128))
                for w_sb, dst, stag in ((wq8, qT8, "sq"), (wk8, kT8, "sk")):
                    stage = stgpool.tile([128, HPC // 2, 512], F8, tag=stag)
                    for o in range(HPC // 2):
                        pq = ps_x.tile([128, 512], F32, tag="pp")
                        for c2 in range(CCH // 2):
                            nc.tensor.matmul(pq[:, :],
                                             w_sb[:, 2 * c2:2 * c2 + 2, 128 * o:128 * (o + 1)],
                                             xs8[:, 2 * c2:2 * c2 + 2, :],
                                             start=(c2 == 0), stop=(c2 == CCH // 2 - 1),
                                             perf_mode=DR)
                        # fp8 eviction on the ACT engine
                        nc.scalar.activation(stage[:, o, :], pq[:, :], AF.Copy)
                    # staging row 64*sh+32*hh+d32 at slot o holds head 2o+hh,
                    # d=32*sh+d32 (host permuted W cols); move to partition
                    # 32*hh+d32, slot [sh, o]
                    for sh in range(2):
                        nc.sync.dma_start(
                            dst[:, sh, :, 512 * s:512 * (s + 1)],
                            stage[64 * sh:64 * sh + 64, :, :])
                for tt in range(4):
                    pv = ps_x.tile([128, 512], F32, tag="pp")
                    for c in range(CCH):
                        nc.tensor.matmul(pv[:, :], xs[:, c, 128 * tt:128 * (tt + 1)],
                                         wv[:, c, :], start=(c == 0), stop=(c == CCH - 1))
                    tv = tmp1.tile([128, 512], F32, tag="t1")
                    nc.vector.tensor_copy(tv[:, :], pv[:, :])
                    # host permuted wv cols to (d, h), so this is a contiguous
                    # 2KB-per-partition move
                    nc.sync.dma_start(
                        v[:, 4 * s + tt, 0:D, :],
                        tv[:, :].rearrange("p (d h) -> p d h", h=HPC).bitcast(F32R))
                if s == 0:
                    nc.sync.dma_start(wp[:, :, :],
                                      wp_d[:, :].rearrange("(c p) o -> p c o", p=128))

            # deferred work queues: each hp's normalize chain (PE bc matmul
            # gated on a DVE copy+reciprocal of its AV PSUM) and each j-block's
            # output projection are emitted later, interleaved into the next
            # hp / j-block, so the PE never sits directly behind those chains
            pending_norm = []
            oproj_queue = []

            def flush_norms():
                for fn in pending_norm:
                    fn()
                pending_norm.clear()

            def emit_oproj_tt(tt):
                ob = out_pool.tile([128, C], F32, tag="ob")
                for o2 in range(2):
                    po = ps_x.tile([128, 512], F32, tag="pp")
                    for c4 in range(GC // 128):
                        nc.tensor.matmul(po[:, :], avT[:, c4, 128 * tt:128 * (tt + 1)],
                                         wp[:, c4, 512 * o2:512 * (o2 + 1)],
                                         start=(c4 == 0), stop=(c4 == GC // 128 - 1))
                    nc.vector.tensor_copy(ob[:, 512 * o2:512 * (o2 + 1)], po[:, :])
                nc.sync.dma_start(out_d[128 * tt:128 * (tt + 1), :], ob[:, :])

            def emit_jblock(j):
                for hp in range(HPC // 2):
                    nkc = 4 * (j + 1)
                    # both heads of the pair run interleaved: their scores
                    # matmuls sit in adjacent PE slots with disjoint row
                    # groups (K=64 at partition 0 vs 64); the pair shares one
                    # 2-bank PSUM tile so a single strided exp covers both
                    av0 = ps_av.tile([65, 512], F32, tag="av")
                    av1 = ps_av.tile([65, 512], F32, tag="av")
                    avs = [av0, av1]
                    def emit_scores_exp(i):
                        sps = ps_s.tile([128, 2, 512], F32, tag="s")
                        roff = 128 * i - 512 * j
                        diag = roff >= 0
                        roff = max(roff, 0)
                        for par in range(2):
                            h = 2 * hp + par
                            hb, hq = h % 2, h // 2
                            nc.tensor.matmul(
                                sps[:, par, roff:512],
                                kT8[32 * hb:32 * hb + 32, :, hq, 128 * i:128 * (i + 1)],
                                qT8[32 * hb:32 * hb + 32, :, hq, 512 * j + roff:512 * (j + 1)],
                                start=True, stop=True, perf_mode=DR)
                        ptile = pt_pool.tile([128, 2, 512], F32R, tag="pt")
                        if diag:
                            # add -1e9 above the diagonal for both pars at once
                            nc.vector.tensor_tensor(
                                sps[:, :, roff:roff + 128], sps[:, :, roff:roff + 128],
                                maskneg[:, :, :], op=mybir.AluOpType.add)
                        # one exp for the par pair; cols [0:roff] are fully
                        # masked and skipped outright
                        nc.scalar.activation(ptile[:, :, roff:512], sps[:, :, roff:512],
                                             AF.Exp, scale=EXP_SCALE)
                        return ptile, roff

                    def emit_av(i, pts):
                        ptile, roff = pts
                        for par in range(2):
                            nc.tensor.matmul(avs[par][:, roff:512], v[:, i, :, 2 * hp + par],
                                             ptile[:, par, roff:512],
                                             start=(i == 0), stop=(i == nkc - 1))

                    # one-chunk software pipeline: chunk i+1's scores sit ahead
                    # of chunk i's AV matmuls in the PE stream, so AV never
                    # waits out the exp latency. After the pipeline is primed,
                    # flush the previous hp's normalize and one deferred
                    # out-proj tile into the PE stream to fill exp-bound gaps.
                    prev = emit_scores_exp(0)
                    for i in range(1, nkc):
                        cur = emit_scores_exp(i)
                        emit_av(i - 1, prev)
                        prev = cur
                        if i == 1:
                            flush_norms()
                        elif i == 3 and oproj_queue:
                            emit_oproj_tt(oproj_queue.pop(0))
                    emit_av(nkc - 1, prev)

                    # eager half of the normalize: the DVE copy frees the AV
                    # PSUM banks for the next hp, and the bf16 reciprocal row
                    # is ready long before the deferred PE broadcast runs
                    for par in range(2):
                        avr = avtmp_pool.tile([65, 512], F32, tag="avr")
                        nc.vector.tensor_copy(avr[:, :], avs[par][:, :])
                        rcp = avtmp_pool.tile([65, 512], BF16, tag="rcp")
                        with nc.allow_low_precision(reason="bf16 denom recip"):
                            nc.vector.reciprocal(rcp[64:65, :], avr[64:65, :])

                        def norm(avr=avr, rcp=rcp, par=par, hp=hp, j=j):
                            # broadcast 1/denom to 128 partitions via K=1
                            # matmul, scale, and ship to avT
                            bc = ps_x.tile([128, 512], F32, tag="pp")
                            nc.tensor.matmul(bc[:, :], ones[64:65, :], rcp[64:65, :],
                                             start=True, stop=True)
                            avf = avtmp_pool.tile([64, 512], F32, tag="avf")
                            nc.vector.tensor_tensor(avf[:, :], avr[0:64, :], bc[0:64, :],
                                                    op=MULT)
                            nc.sync.dma_start(
                                avT[64 * par:64 * par + 64, hp, 512 * j:512 * (j + 1)],
                                avf[:, :].bitcast(F32R))
                        pending_norm.append(norm)

                oproj_queue.extend(range(4 * j, 4 * (j + 1)))

            for s in range(NSTRIP):
                emit_strip(s)
                emit_jblock(s)
            flush_norms()
            while oproj_queue:
                emit_oproj_tt(oproj_queue.pop(0))
    _split_excess_waits(nc)
    return nc


def _get_program():
    global _PROGRAM
    if _PROGRAM is None:
        _PROGRAM = _build_program()
    return _PROGRAM


def _make_in_maps(x, Wk, Wq, Wv, Wp):
    import ml_dtypes
    f8 = ml_dtypes.float8_e4m3
    x = np.asarray(x, dtype=np.float32)
    Wk = np.asarray(Wk, dtype=np.float32)
    Wq = np.asarray(Wq, dtype=np.float32)
    Wv = np.asarray(Wv, dtype=np.float32)
    Wp = np.asarray(Wp, dtype=np.float32)
    # fp8 weight column permutation: col 128*o+64*sh+32*hh+d32 holds original
    # out-channel (2*o+hh)*64 + 32*sh + d32, so the q/k eviction DMA is a pure
    # partition-offset move into the [32*(h%4)+d32, d//32, h//4, t] layout
    perm = np.empty(GC, np.int64)
    for o in range(4):
        for sh in range(2):
            for hh in range(2):
                base = 128 * o + 64 * sh + 32 * hh
                perm[base:base + 32] = (2 * o + hh) * 64 + 32 * sh + np.arange(32)
    # wv column permutation: col d*8+h holds original out-channel h*64+d, so
    # the v eviction writes d-major contiguous runs
    perm_v = np.empty(GC, np.int64)
    for d in range(D):
        perm_v[d * HPC:(d + 1) * HPC] = np.arange(HPC) * D + d
    in_maps = []
    for core in range(8):
        b, g = core // GROUPS, core % GROUPS
        rows = slice(GC * g, GC * (g + 1))
        xT = np.ascontiguousarray(x[b].T)
        in_maps.append({
            "xT": xT,                                           # [C, T]
            "xT8": np.ascontiguousarray(xT.astype(f8)),
            "wq8": np.ascontiguousarray((WSCALE * Wq[rows, :][perm, :].T).astype(f8)),
            "wk8": np.ascontiguousarray((WSCALE * Wk[rows, :][perm, :].T).astype(f8)),
            "wvT": np.ascontiguousarray(Wv[rows, :][perm_v, :].T),
            "wpT": np.ascontiguousarray(Wp[:, rows].T),         # [GC, C]
        })
    return in_maps


def run(x, Wk, Wq, Wv, Wp, bp, trace=False, **spmd_kwargs):
    nc = _get_program()
    in_maps = _make_in_maps(x, Wk, Wq, Wv, Wp)
    res = run_bass_kernel_spmd(nc, in_maps, list(range(8)), trace=trace, **spmd_kwargs)
    bp = np.asarray(bp, dtype=np.float32)
    out = np.empty((B, T, C), dtype=np.float32)
    for b in range(B):
        out[b] = res.results[GROUPS * b]["outp"] + res.results[GROUPS * b + 1]["outp"] + bp
    return out, res


def kernel(x, Wk, Wq, Wv, Wp, bp):
    out, _ = run(x, Wk, Wq, Wv, Wp, bp)
    return out



# revision 42
# speedup vs baseline: 1.3374x; 1.0322x over previous
"""Multi-head causal self-attention (B=4, T=2048, C=1024, 16 heads) on 8 trn2 cores.

Sharding: data-parallel over batch (4) x tensor-parallel over heads (2 groups of 8).
Core m handles batch m//2, head group m%2. Host pre-transposes x and the weights so
every on-device matmul consumes operands in natural layout (zero on-device
transposes); the output projection partial sums are pair-reduced on host (+bias).

Per-core pipeline (all matmuls fp32r = FP22 multiply, fp32 PSUM accumulate):
  qT[o,t] = Wq_g @ x^T        (lhsT = Wq_g^T chunks, rhs = x^T chunks)
  kT[o,t] likewise; v[t,o]    (lhsT = x^T chunks, rhs = Wv_g^T)
  scores^T[k,q] per head      (lhsT = kT tile [64,128], rhs = qT tile [64,512])
  p = exp(0.125*scores^T)     (ACT, causal mask via memset + triangle multiply)
  [AV^T | denom] = [v|1]^T @ p (ones column of v gives softmax denominators)
  avT = AV^T * exp(-ln(denom)) broadcast via K=1 ones-matmul
  out_partial = avT^T @ Wp_g^T
"""

import numpy as np

import concourse.bass as bass
import concourse.mybir as mybir
import concourse.tile as tile
from concourse.bass_utils import run_bass_kernel_spmd

F32 = mybir.dt.float32
F32R = mybir.dt.float32r
F8 = mybir.dt.float8e4
BF16 = mybir.dt.bfloat16
DR = mybir.MatmulPerfMode.DoubleRow
AF = mybir.ActivationFunctionType
MULT = mybir.AluOpType.mult
WSCALE = 32.0               # host scales Wq/Wk by 32 so fp8 stays out of subnormals
EXP_SCALE = 0.125 / (WSCALE * WSCALE)

B, T, C = 4, 2048, 1024
HEADS, D = 16, 64
GROUPS = 2                  # head groups (tensor parallel)
HPC = HEADS // GROUPS       # heads per core = 8
GC = HPC * D                # group channel width = 512
NKC = T // 128              # Tk chunks = 16
NJ = T // 512               # Tq tiles = 4
CCH = C // 128              # contraction chunks = 8
NSTRIP = T // 512           # phase-1 t strips = 4

_PROGRAM = None


def _patch_drain_chunking():
    """The axon walrus build rejects instructions with >~4 sem waits; Tile's
    kernel-tail drain waits on every live semaphore at once. Split it into a
    chain of drains with <=2 waits each."""
    from bass_rust import VectorClock, ScopedClock

    if getattr(tile.TileContext, "_drain_chunk_patched", False):
        return

    def _drain_and_barrier(self, tick_clock, wait_clock):
        gc_vec = list(tick_clock.global_clock)
        nz = [i for i, t in enumerate(gc_vec) if t > 0]
        CHUNK = 1
        for k in range(0, len(nz), CHUNK):
            keep = set(nz[k:k + CHUNK])
            partial = [gc_vec[i] if i in keep else 0 for i in range(len(gc_vec))]
            d = self.nc.sync.drain()
            wait_clock.add_sem_waits(d.ins, ScopedClock({None: VectorClock(partial)}))
        self.nc.all_engine_barrier()
        assert self.sems is not None
        popped = self.nc._tile_sem_poison_stack.pop()
        assert popped is self._sem_poison
        self.nc.clear_and_free_semaphores(list(self.sems.allocated().values()))
        self.nc.all_engine_barrier()

    tile.TileContext._drain_and_barrier = _drain_and_barrier
    tile.TileContext._drain_chunk_patched = True


def _split_excess_waits(nc, maxw=1, maxw_other=None):
    """Walrus rejects instructions carrying more than ~1 sem wait (proven for
    PE matmul S3_LW and the SP drain at 5). Move excess waits onto same-engine
    NoOps inserted immediately before the instruction (engine streams execute
    in bb order, so semantics are preserved). maxw_other, if set, applies to
    non-PE engines."""
    from bass_rust import InstNoOp

    ctr = 0
    for f in nc.m.functions:
        for bb in f.blocks:
            new_insts = []
            for inst in bb.instructions:
                si = inst.sync_info
                waits = list(si.on_wait) if si and si.on_wait else []
                lim = maxw
                if maxw_other is not None and str(inst.engine) != 'EngineType.PE':
                    lim = maxw_other
                maxw_eff = lim
                if len(waits) > maxw_eff:
                    head, rest = waits[:-maxw_eff], waits[-maxw_eff:]
                    for k in range(0, len(head), maxw_eff):
                        ctr += 1
                        new_insts.append(InstNoOp(
                            name=f"waitnop_{ctr}",
                            engine=inst.engine,
                            sync_info=mybir.SyncInfo(
                                on_wait=head[k:k + maxw_eff], on_update=[]),
                        ))
                    inst.sync_info = mybir.SyncInfo(on_wait=rest, on_update=si.on_update)
                new_insts.append(inst)
            bb.instructions = new_insts
    return ctr


def _build_program():
    _patch_drain_chunking()
    nc = bass.Bass()

    xT_d = nc.declare_dram_parameter("xT", [C, T], F32R, isOutput=False)
    xT8_d = nc.declare_dram_parameter("xT8", [C, T], F8, isOutput=False)
    wq8_d = nc.declare_dram_parameter("wq8", [C, GC], F8, isOutput=False)
    wk8_d = nc.declare_dram_parameter("wk8", [C, GC], F8, isOutput=False)
    wv_d = nc.declare_dram_parameter("wvT", [C, GC], F32R, isOutput=False)
    wp_d = nc.declare_dram_parameter("wpT", [GC, C], F32R, isOutput=False)
    out_d = nc.declare_dram_parameter("outp", [T, C], F32, isOutput=True)

    from contextlib import ExitStack

    with tile.TileContext(nc) as tc, ExitStack() as stack:
        cpool = stack.enter_context(tc.tile_pool(name="const", bufs=1))
        qkv_pool = stack.enter_context(tc.tile_pool(name="qkv", bufs=1))

        # additive causal mask: 0 where q >= k, -1e9 where q < k (exp -> 0),
        # replicated on both par slots so one strided DVE add masks the pair
        maskneg = cpool.tile([128, 2, 128], F32)
        nc.gpsimd.memset(maskneg[:, :, :], 0.0)
        for s in range(2):
            nc.gpsimd.affine_select(
                out=maskneg[:, s, :], in_=maskneg[:, s, :],
                compare_op=mybir.AluOpType.is_ge, fill=-1e9, base=0,
                pattern=[[1, 128]], channel_multiplier=-1,
            )
        # DVE cannot encode f32r, so f32r tiles are written by ACT/DMA only.
        # ones row is bf16 so the K=1 broadcast matmul can take a DVE-written
        # bf16 rhs (DVE cannot write f32r; verifier demands rounded operands)
        BF16 = mybir.dt.bfloat16
        ones = cpool.tile([128, 128], BF16)
        nc.scalar.activation(ones[64:65, :], ones[64:65, :], AF.Copy, scale=0.0, bias=1.0)

        # fp8 q/k for DoubleRow scores: partition = 32*(h%2) + d%32 (PE operand
        # base partition must be 0/32/64), free = [d//32 (stacked contraction
        # half), h//2, t]
        qT8 = qkv_pool.tile([64, 2, HPC // 2, T], F8)
        kT8 = qkv_pool.tile([64, 2, HPC // 2, T], F8)
        # v padded with a ones row per head: [t-chunk, d, head] (d-major via
        # host wv column permutation so the eviction DMA moves 2KB-contiguous
        # runs); row 64 = ones for the softmax denominators
        v = qkv_pool.tile([128, NKC, D + 1, HPC], F32R)
        nc.scalar.activation(v[:, :, D, :], v[:, :, D, :],
                             AF.Copy, scale=0.0, bias=1.0)

        avT = qkv_pool.tile([128, HPC // 2, T], F32R)
        wp = qkv_pool.tile([128, GC // 128, C], F32R)

        # Interleaved schedule: phase-1 strip s immediately followed by the
        # attention j-block j=s (its queries live in strip s; its keys/values
        # in strips 0..s), so the SP-bound projection DMAs and the ACT-bound
        # exp stream overlap instead of running as separate phases.
        # PSUM budget (8 banks): ps_s 2x2 + ps_av 2 + ps_x 2 (shared by the
        # qkv projection accumulators, the denom broadcast, and out-proj).
        with tc.tile_pool(name="w1", bufs=1) as wpool, \
             tc.tile_pool(name="xs", bufs=2) as xpool, \
             tc.tile_pool(name="xs8", bufs=1) as x8pool, \
             tc.tile_pool(name="stg", bufs=1) as stgpool, \
             tc.tile_pool(name="tmp1", bufs=2) as tmp1, \
             tc.tile_pool(name="pt", bufs=2) as pt_pool, \
             tc.tile_pool(name="avtmp", bufs=2) as avtmp_pool, \
             tc.tile_pool(name="ob", bufs=2) as out_pool, \
             tc.tile_pool(name="ps_s", bufs=2, space="PSUM") as ps_s, \
             tc.tile_pool(name="ps_av", bufs=2, space="PSUM") as ps_av, \
             tc.tile_pool(name="ps_x", bufs=2, space="PSUM") as ps_x:
            wq8 = wpool.tile([128, CCH, GC], F8)
            wk8 = wpool.tile([128, CCH, GC], F8)
            wv = wpool.tile([128, CCH, GC], F32R)

            def emit_strip(s):
                xs8 = x8pool.tile([128, CCH, 512], F8)
                nc.sync.dma_start(
                    xs8[:, :, :],
                    xT8_d[:, 512 * s:512 * (s + 1)].rearrange("(c p) t -> p c t", p=128))
                if s == 0:
                    # small fp8 weight loads right after the (small) fp8 x strip
                    # so the first Q DoubleRow matmul starts ~4us in; the big
                    # f32r wv/x-strip transfers follow (V matmuls run last)
                    for w_sb, w_d in ((wq8, wq8_d), (wk8, wk8_d)):
                        nc.sync.dma_start(w_sb[:, :, :],
                                          w_d[:, :].rearrange("(c p) o -> p c o", p=128))
                    nc.sync.dma_start(wv[:, :, :],
                                      wv_d[:, :].rearrange("(c p) o -> p c o", p=128))
                # split the f32r strip so V's first contraction chunks can
                # start before the whole strip lands
                xs = xpool.tile([128, CCH, 512], F32R)
                for ch in range(2):
                    cs = slice(4 * ch, 4 * (ch + 1))
                    nc.sync.dma_start(
                        xs[:, cs, :],
                        xT_d[128 * cs.start:128 * cs.stop,
                             512 * s:512 * (s + 1)].rearrange("(c p) t -> p c t", p=128))
                for w_sb, dst, stag in ((wq8, qT8, "sq"), (wk8, kT8, "sk")):
                    stage = stgpool.tile([128, HPC // 2, 512], F8, tag=stag)
                    for o in range(HPC // 2):
                        pq = ps_x.tile([128, 512], F32, tag="pp")
                        for c2 in range(CCH // 2):
                            nc.tensor.matmul(pq[:, :],
                                             w_sb[:, 2 * c2:2 * c2 + 2, 128 * o:128 * (o + 1)],
                                             xs8[:, 2 * c2:2 * c2 + 2, :],
                                             start=(c2 == 0), stop=(c2 == CCH // 2 - 1),
                                             perf_mode=DR)
                        # fp8 eviction on DVE (keeps the ACT engine free for
                        # the exp stream of the interleaved j-blocks)
                        nc.vector.tensor_copy(stage[:, o, :], pq[:, :])
                    # staging row 64*sh+32*hh+d32 at slot o holds head 2o+hh,
                    # d=32*sh+d32 (host permuted W cols); move to partition
                    # 32*hh+d32, slot [sh, o]
                    for sh in range(2):
                        nc.sync.dma_start(
                            dst[:, sh, :, 512 * s:512 * (s + 1)],
                            stage[64 * sh:64 * sh + 64, :, :])
                for tt in range(4):
                    pv = ps_x.tile([128, 512], F32, tag="pp")
                    for c in range(CCH):
                        nc.tensor.matmul(pv[:, :], xs[:, c, 128 * tt:128 * (tt + 1)],
                                         wv[:, c, :], start=(c == 0), stop=(c == CCH - 1))
                    tv = tmp1.tile([128, 512], F32, tag="t1")
                    nc.vector.tensor_copy(tv[:, :], pv[:, :])
                    # host permuted wv cols to (d, h), so this is a contiguous
                    # 2KB-per-partition move
                    nc.sync.dma_start(
                        v[:, 4 * s + tt, 0:D, :],
                        tv[:, :].rearrange("p (d h) -> p d h", h=HPC).bitcast(F32R))
                if s == 2:
                    # wp is first consumed by the deferred out-proj of jb0
                    # (inside jb1); loading it here keeps the DMA device free
                    # for the latency-critical early strip traffic
                    nc.sync.dma_start(wp[:, :, :],
                                      wp_d[:, :].rearrange("(c p) o -> p c o", p=128))

            # deferred work queues: each hp's normalize chain (PE bc matmul
            # gated on a DVE copy+reciprocal of its AV PSUM) and each j-block's
            # output projection are emitted later, interleaved into the next
            # hp / j-block, so the PE never sits directly behind those chains
            pending_norm = []
            oproj_queue = []

            def flush_norms():
                for fn in pending_norm:
                    fn()
                pending_norm.clear()

            def emit_oproj_tt(tt):
                ob = out_pool.tile([128, C], F32, tag="ob")
                for o2 in range(2):
                    po = ps_x.tile([128, 512], F32, tag="pp")
                    for c4 in range(GC // 128):
                        nc.tensor.matmul(po[:, :], avT[:, c4, 128 * tt:128 * (tt + 1)],
                                         wp[:, c4, 512 * o2:512 * (o2 + 1)],
                                         start=(c4 == 0), stop=(c4 == GC // 128 - 1))
                    nc.vector.tensor_copy(ob[:, 512 * o2:512 * (o2 + 1)], po[:, :])
                nc.sync.dma_start(out_d[128 * tt:128 * (tt + 1), :], ob[:, :])

            def emit_jblock(j):
                nkc = 4 * (j + 1)
                avs_by_hp = {}

                def emit_scores_exp(hp, i):
                    # both heads of the pair run interleaved; the pair shares
                    # one 2-bank PSUM tile so a single strided exp covers both
                    sps = ps_s.tile([128, 2, 512], F32, tag="s")
                    roff = 128 * i - 512 * j
                    diag = roff >= 0
                    roff = max(roff, 0)
                    for par in range(2):
                        h = 2 * hp + par
                        hb, hq = h % 2, h // 2
                        nc.tensor.matmul(
                            sps[:, par, roff:512],
                            kT8[32 * hb:32 * hb + 32, :, hq, 128 * i:128 * (i + 1)],
                            qT8[32 * hb:32 * hb + 32, :, hq, 512 * j + roff:512 * (j + 1)],
                            start=True, stop=True, perf_mode=DR)
                    ptile = pt_pool.tile([128, 2, 512], F32R, tag="pt")
                    if diag:
                        # add -1e9 above the diagonal for both pars at once
                        nc.vector.tensor_tensor(
                            sps[:, :, roff:roff + 128], sps[:, :, roff:roff + 128],
                            maskneg[:, :, :], op=mybir.AluOpType.add)
                    # one exp for the par pair; cols [0:roff] are fully
                    # masked and skipped outright
                    nc.scalar.activation(ptile[:, :, roff:512], sps[:, :, roff:512],
                                         AF.Exp, scale=EXP_SCALE)
                    return ptile, roff

                def emit_av(hp, i, pts):
                    ptile, roff = pts
                    if i == 0:
                        av0 = ps_av.tile([65, 512], F32, tag="av")
                        av1 = ps_av.tile([65, 512], F32, tag="av")
                        avs_by_hp[hp] = [av0, av1]
                    for par in range(2):
                        nc.tensor.matmul(avs_by_hp[hp][par][:, roff:512],
                                         v[:, i, :, 2 * hp + par],
                                         ptile[:, par, roff:512],
                                         start=(i == 0), stop=(i == nkc - 1))

                def post_hp(hp):
                    # eager half of the normalize: the DVE copies free the AV
                    # PSUM banks for the next hp, and the bf16 reciprocal rows
                    # are ready long before the deferred PE broadcast runs
                    avs = avs_by_hp.pop(hp)
                    for par in range(2):
                        avr = avtmp_pool.tile([65, 512], F32, tag="avr")
                        nc.vector.tensor_copy(avr[:, :], avs[par][:, :])
                        rcp = avtmp_pool.tile([65, 512], BF16, tag="rcp")
                        with nc.allow_low_precision(reason="bf16 denom recip"):
                            nc.vector.reciprocal(rcp[64:65, :], avr[64:65, :])

                        def norm(avr=avr, rcp=rcp, par=par, hp=hp, j=j):
                            bc = ps_x.tile([128, 512], F32, tag="pp")
                            nc.tensor.matmul(bc[:, :], ones[64:65, :], rcp[64:65, :],
                                             start=True, stop=True)
                            avf = avtmp_pool.tile([64, 512], F32, tag="avf")
                            nc.vector.tensor_tensor(avf[:, :], avr[0:64, :],
                                                    bc[0:64, :], op=MULT)
                            nc.sync.dma_start(
                                avT[64 * par:64 * par + 64, hp, 512 * j:512 * (j + 1)],
                                avf[:, :].bitcast(F32R))
                        pending_norm.append(norm)

                # one-chunk software pipeline carried ACROSS hp boundaries:
                # the next group's scores always sit ahead of the previous
                # group's AV matmuls in the PE stream, so the ACT exp stream
                # never waits at an hp seam. Deferred normalize and out-proj
                # tiles slot in as PE fill during the exp-bound stretches.
                prev = None
                for hp in range(HPC // 2):
                    for i in range(nkc):
                        cur = (hp, i, emit_scores_exp(hp, i))
                        if i == 1:
                            flush_norms()
                        elif i == 3 and oproj_queue:
                            emit_oproj_tt(oproj_queue.pop(0))
                        if prev is not None:
                            php, pi, pts = prev
                            emit_av(php, pi, pts)
                            if pi == nkc - 1:
                                post_hp(php)
                        prev = cur
                php, pi, pts = prev
                emit_av(php, pi, pts)
                post_hp(php)

                oproj_queue.extend(range(4 * j, 4 * (j + 1)))

            # strips run ahead of the attention blocks so the ACT exp stream
            # (the binding engine of the attention stretches) starts early and
            # never starves; j-blocks then run back-to-back with strip/out-proj
            # matmuls as PE fill during the exp-bound stretches
            emit_strip(0)
            emit_strip(1)
            emit_jblock(0)
            emit_strip(2)
            emit_jblock(1)
            emit_strip(3)
            emit_jblock(2)
            emit_jblock(3)
            flush_norms()
            while oproj_queue:
                emit_oproj_tt(oproj_queue.pop(0))
    _split_excess_waits(nc)
    return nc


def _get_program():
    global _PROGRAM
    if _PROGRAM is None:
        _PROGRAM = _build_program()
    return _PROGRAM


def _make_in_maps(x, Wk, Wq, Wv, Wp):
    import ml_dtypes
    f8 = ml_dtypes.float8_e4m3
    x = np.asarray(x, dtype=np.float32)
    Wk = np.asarray(Wk, dtype=np.float32)
    Wq = np.asarray(Wq, dtype=np.float32)
    Wv = np.asarray(Wv, dtype=np.float32)
    Wp = np.asarray(Wp, dtype=np.float32)
    # fp8 weight column permutation: col 128*o+64*sh+32*hh+d32 holds original
    # out-channel (2*o+hh)*64 + 32*sh + d32, so the q/k eviction DMA is a pure
    # partition-offset move into the [32*(h%4)+d32, d//32, h//4, t] layout
    perm = np.empty(GC, np.int64)
    for o in range(4):
        for sh in range(2):
            for hh in range(2):
                base = 128 * o + 64 * sh + 32 * hh
                perm[base:base + 32] = (2 * o + hh) * 64 + 32 * sh + np.arange(32)
    # wv column permutation: col d*8+h holds original out-channel h*64+d, so
    # the v eviction writes d-major contiguous runs
    perm_v = np.empty(GC, np.int64)
    for d in range(D):
        perm_v[d * HPC:(d + 1) * HPC] = np.arange(HPC) * D + d
    in_maps = []
    for core in range(8):
        b, g = core // GROUPS, core % GROUPS
        rows = slice(GC * g, GC * (g + 1))
        xT = np.ascontiguousarray(x[b].T)
        in_maps.append({
            "xT": xT,                                           # [C, T]
            "xT8": np.ascontiguousarray(xT.astype(f8)),
            "wq8": np.ascontiguousarray((WSCALE * Wq[rows, :][perm, :].T).astype(f8)),
            "wk8": np.ascontiguousarray((WSCALE * Wk[rows, :][perm, :].T).astype(f8)),
            "wvT": np.ascontiguousarray(Wv[rows, :][perm_v, :].T),
            "wpT": np.ascontiguousarray(Wp[:, rows].T),         # [GC, C]
        })
    return in_maps


def run(x, Wk, Wq, Wv, Wp, bp, trace=False, **spmd_kwargs):
    nc = _get_program()
    in_maps = _make_in_maps(x, Wk, Wq, Wv, Wp)
    res = run_bass_kernel_spmd(nc, in_maps, list(range(8)), trace=trace, **spmd_kwargs)
    bp = np.asarray(bp, dtype=np.float32)
    out = np.empty((B, T, C), dtype=np.float32)
    for b in range(B):
        out[b] = res.results[GROUPS * b]["outp"] + res.results[GROUPS * b + 1]["outp"] + bp
    return out, res


def kernel(x, Wk, Wq, Wv, Wp, bp):
    out, _ = run(x, Wk, Wq, Wv, Wp, bp)
    return out



# revision 54
# speedup vs baseline: 1.4205x; 1.0621x over previous
"""Multi-head causal self-attention (B=4, T=2048, C=1024, 16 heads) on 8 trn2 cores.

Sharding: data-parallel over batch (4) x tensor-parallel over heads (2 groups of 8).
Core m handles batch m//2, head group m%2. Host pre-transposes x and the weights so
every on-device matmul consumes operands in natural layout (zero on-device
transposes); the output projection partial sums are pair-reduced on host (+bias).

Per-core pipeline (all matmuls fp32r = FP22 multiply, fp32 PSUM accumulate):
  qT[o,t] = Wq_g @ x^T        (lhsT = Wq_g^T chunks, rhs = x^T chunks)
  kT[o,t] likewise; v[t,o]    (lhsT = x^T chunks, rhs = Wv_g^T)
  scores^T[k,q] per head      (lhsT = kT tile [64,128], rhs = qT tile [64,512])
  p = exp(0.125*scores^T)     (ACT, causal mask via memset + triangle multiply)
  [AV^T | denom] = [v|1]^T @ p (ones column of v gives softmax denominators)
  avT = AV^T * exp(-ln(denom)) broadcast via K=1 ones-matmul
  out_partial = avT^T @ Wp_g^T
"""

import numpy as np

import concourse.bass as bass
import concourse.mybir as mybir
import concourse.tile as tile
from concourse.bass_utils import run_bass_kernel_spmd

F32 = mybir.dt.float32
F32R = mybir.dt.float32r
F8 = mybir.dt.float8e4
BF16 = mybir.dt.bfloat16
DR = mybir.MatmulPerfMode.DoubleRow
AF = mybir.ActivationFunctionType
MULT = mybir.AluOpType.mult
WSCALE = 32.0               # host scales Wq/Wk by 32 so fp8 stays out of subnormals
EXP_SCALE = 0.125 / (WSCALE * WSCALE)

B, T, C = 4, 2048, 1024
HEADS, D = 16, 64
GROUPS = 2                  # head groups (tensor parallel)
HPC = HEADS // GROUPS       # heads per core = 8
GC = HPC * D                # group channel width = 512
NKC = T // 128              # Tk chunks = 16
NJ = T // 512               # Tq tiles = 4
CCH = C // 128              # contraction chunks = 8
NSTRIP = T // 512           # phase-1 t strips = 4

_PROGRAM = None


def _patch_drain_chunking():
    """The axon walrus build rejects instructions with >~4 sem waits; Tile's
    kernel-tail drain waits on every live semaphore at once. Split it into a
    chain of drains with <=2 waits each."""
    from bass_rust import VectorClock, ScopedClock

    if getattr(tile.TileContext, "_drain_chunk_patched", False):
        return

    def _drain_and_barrier(self, tick_clock, wait_clock):
        gc_vec = list(tick_clock.global_clock)
        nz = [i for i, t in enumerate(gc_vec) if t > 0]
        CHUNK = 1
        for k in range(0, len(nz), CHUNK):
            keep = set(nz[k:k + CHUNK])
            partial = [gc_vec[i] if i in keep else 0 for i in range(len(gc_vec))]
            d = self.nc.sync.drain()
            wait_clock.add_sem_waits(d.ins, ScopedClock({None: VectorClock(partial)}))
        self.nc.all_engine_barrier()
        assert self.sems is not None
        popped = self.nc._tile_sem_poison_stack.pop()
        assert popped is self._sem_poison
        self.nc.clear_and_free_semaphores(list(self.sems.allocated().values()))
        self.nc.all_engine_barrier()

    tile.TileContext._drain_and_barrier = _drain_and_barrier
    tile.TileContext._drain_chunk_patched = True


def _split_excess_waits(nc, maxw=1, maxw_other=None):
    """Walrus rejects instructions carrying more than ~1 sem wait (proven for
    PE matmul S3_LW and the SP drain at 5). Move excess waits onto same-engine
    NoOps inserted immediately before the instruction (engine streams execute
    in bb order, so semantics are preserved). maxw_other, if set, applies to
    non-PE engines."""
    from bass_rust import InstNoOp

    ctr = 0
    for f in nc.m.functions:
        for bb in f.blocks:
            new_insts = []
            for inst in bb.instructions:
                si = inst.sync_info
                waits = list(si.on_wait) if si and si.on_wait else []
                lim = maxw
                if maxw_other is not None and str(inst.engine) != 'EngineType.PE':
                    lim = maxw_other
                maxw_eff = lim
                if len(waits) > maxw_eff:
                    head, rest = waits[:-maxw_eff], waits[-maxw_eff:]
                    for k in range(0, len(head), maxw_eff):
                        ctr += 1
                        new_insts.append(InstNoOp(
                            name=f"waitnop_{ctr}",
                            engine=inst.engine,
                            sync_info=mybir.SyncInfo(
                                on_wait=head[k:k + maxw_eff], on_update=[]),
                        ))
                    inst.sync_info = mybir.SyncInfo(on_wait=rest, on_update=si.on_update)
                new_insts.append(inst)
            bb.instructions = new_insts
    return ctr


def _build_program():
    _patch_drain_chunking()
    nc = bass.Bass()

    xT_d = nc.declare_dram_parameter("xT", [C, T], F32R, isOutput=False)
    xT8_d = nc.declare_dram_parameter("xT8", [C, T], F8, isOutput=False)
    wq8_d = nc.declare_dram_parameter("wq8", [C, GC], F8, isOutput=False)
    wk8_d = nc.declare_dram_parameter("wk8", [C, GC], F8, isOutput=False)
    wv_d = nc.declare_dram_parameter("wvT", [C, GC], F32R, isOutput=False)
    wp_d = nc.declare_dram_parameter("wpT", [GC, C], F32R, isOutput=False)
    out_d = nc.declare_dram_parameter("outp", [T, C], F32, isOutput=True)

    from contextlib import ExitStack

    with tile.TileContext(nc) as tc, ExitStack() as stack:
        cpool = stack.enter_context(tc.tile_pool(name="const", bufs=1))
        qkv_pool = stack.enter_context(tc.tile_pool(name="qkv", bufs=1))

        # additive causal mask: 0 where q >= k, -1e9 where q < k (exp -> 0),
        # replicated on both par slots so one strided DVE add masks the pair
        maskneg = cpool.tile([128, 2, 128], F32)
        nc.gpsimd.memset(maskneg[:, :, :], 0.0)
        for s in range(2):
            nc.gpsimd.affine_select(
                out=maskneg[:, s, :], in_=maskneg[:, s, :],
                compare_op=mybir.AluOpType.is_ge, fill=-1e9, base=0,
                pattern=[[1, 128]], channel_multiplier=-1,
            )
        # DVE cannot encode f32r, so f32r tiles are written by ACT/DMA only.
        # ones row is bf16 so the K=1 broadcast matmul can take a DVE-written
        # bf16 rhs (DVE cannot write f32r; verifier demands rounded operands)
        BF16 = mybir.dt.bfloat16
        ones = cpool.tile([128, 128], BF16)
        nc.scalar.activation(ones[64:65, :], ones[64:65, :], AF.Copy, scale=0.0, bias=1.0)

        # fp8 q/k for DoubleRow scores: partition = 32*(h%2) + d%32 (PE operand
        # base partition must be 0/32/64), free = [d//32 (stacked contraction
        # half), h//2, t]
        kT8 = qkv_pool.tile([64, 2, HPC // 2, T], F8)
        # v padded with a ones row per head: [t-chunk, d, head] (d-major via
        # host wv column permutation so the eviction DMA moves 2KB-contiguous
        # runs); row 64 = ones for the softmax denominators
        v = qkv_pool.tile([128, NKC, D + 1, HPC], F32R)
        nc.scalar.activation(v[:, :, D, :], v[:, :, D, :],
                             AF.Copy, scale=0.0, bias=1.0)

        avT = qkv_pool.tile([128, HPC // 2, T], F32R)
        wp = qkv_pool.tile([128, GC // 128, C], F32R)

        # q lives in per-strip tiles (jb j only reads strip j's queries)
        # Interleaved schedule: phase-1 strip s immediately followed by the
        # attention j-block j=s (its queries live in strip s; its keys/values
        # in strips 0..s), so the SP-bound projection DMAs and the ACT-bound
        # exp stream overlap instead of running as separate phases.
        # PSUM budget (8 banks): ps_s 2x2 + ps_av 2 + ps_x 2 (shared by the
        # qkv projection accumulators, the denom broadcast, and out-proj).
        with tc.tile_pool(name="w1", bufs=1) as wpool, \
             tc.tile_pool(name="xs", bufs=2) as xpool, \
             tc.tile_pool(name="xs8", bufs=2) as x8pool, \
             tc.tile_pool(name="stg", bufs=1) as stgpool, \
             tc.tile_pool(name="qst", bufs=2) as qpool, \
             tc.tile_pool(name="tmp1", bufs=2) as tmp1, \
             tc.tile_pool(name="pt", bufs=3) as pt_pool, \
             tc.tile_pool(name="avtmp", bufs=2) as avtmp_pool, \
             tc.tile_pool(name="ob", bufs=2) as out_pool, \
             tc.tile_pool(name="ps_s", bufs=2, space="PSUM") as ps_s, \
             tc.tile_pool(name="ps_av", bufs=2, space="PSUM") as ps_av, \
             tc.tile_pool(name="ps_x", bufs=2, space="PSUM") as ps_x:
            wq8 = wpool.tile([128, CCH, GC], F8)
            wk8 = wpool.tile([128, CCH, GC], F8)
            wv = wpool.tile([128, CCH, GC], F32R)

            def emit_strip_loads_a(s):
                # latency-critical small fp8 loads: x strip + (once) q/k weights
                xs8 = x8pool.tile([128, CCH, 512], F8)
                nc.sync.dma_start(
                    xs8[:, :, :],
                    xT8_d[:, 512 * s:512 * (s + 1)].rearrange("(c p) t -> p c t", p=128))
                if s == 0:
                    for w_sb, w_d in ((wq8, wq8_d), (wk8, wk8_d)):
                        nc.sync.dma_start(w_sb[:, :, :],
                                          w_d[:, :].rearrange("(c p) o -> p c o", p=128))
                return xs8

            def emit_strip_loads_b(s):
                # heavy f32r loads for the V path; emitted after the q/k
                # eviction DMAs of the covering strip so those small transfers
                # win the DMA device first
                xs = xpool.tile([128, CCH, 512], F32R)
                cs0 = slice(0, 4)
                nc.sync.dma_start(
                    xs[:, cs0, :],
                    xT_d[0:512, 512 * s:512 * (s + 1)].rearrange("(c p) t -> p c t", p=128))
                if s == 0:
                    nc.sync.dma_start(wv[:, :, :],
                                      wv_d[:, :].rearrange("(c p) o -> p c o", p=128))
                cs1 = slice(4, 8)
                nc.sync.dma_start(
                    xs[:, cs1, :],
                    xT_d[512:1024, 512 * s:512 * (s + 1)].rearrange("(c p) t -> p c t", p=128))
                if s == 2:
                    # wp is first consumed by the deferred out-proj of jb0
                    # (inside jb1)
                    nc.sync.dma_start(wp[:, :, :],
                                      wp_d[:, :].rearrange("(c p) o -> p c o", p=128))
                return xs

            qtiles = {}

            def qk_compute_groups(s, xs8):
                # one closure per q/k projection tile
                groups = []
                stages = {}
                for w_sb, stag in ((wq8, "sq"), (wk8, "sk")):
                    for o in range(HPC // 2):
                        def qk_group(w_sb=w_sb, stag=stag, o=o):
                            if stag not in stages:
                                stage_t = stgpool.tile([128, HPC // 2, 512], F8, tag=stag)
                                stages[stag] = stage_t
                            stage = stages[stag]
                            pq = ps_x.tile([128, 512], F32, tag="pp")
                            for c2 in range(CCH // 2):
                                nc.tensor.matmul(
                                    pq[:, :],
                                    w_sb[:, 2 * c2:2 * c2 + 2, 128 * o:128 * (o + 1)],
                                    xs8[:, 2 * c2:2 * c2 + 2, :],
                                    start=(c2 == 0), stop=(c2 == CCH // 2 - 1),
                                    perf_mode=DR)
                            # fp8 eviction on DVE (keeps ACT free for exps)
                            nc.vector.tensor_copy(stage[:, o, :], pq[:, :])
                            if o == HPC // 2 - 1:
                                # staging row 64*sh+32*hh+d32 at slot o holds
                                # head 2o+hh, d=32*sh+d32 (host permuted W
                                # cols); move to partition 32*hh+d32, [sh, o]
                                if stag == "sq":
                                    qt = qpool.tile([64, 2, HPC // 2, 512], F8, tag="qT8")
                                    qtiles[s] = qt
                                    dst_aps = [qt[:, sh, :, :] for sh in range(2)]
                                else:
                                    dst_aps = [kT8[:, sh, :, 512 * s:512 * (s + 1)]
                                               for sh in range(2)]
                                for sh in range(2):
                                    nc.sync.dma_start(
                                        dst_aps[sh],
                                        stage[64 * sh:64 * sh + 64, :, :])
                        groups.append(qk_group)
                return groups

            def v_compute_groups(s, xs):
                groups = []
                for tt in range(4):
                    def v_group(tt=tt):
                        pv = ps_x.tile([128, 512], F32, tag="pp")
                        for c in range(CCH):
                            nc.tensor.matmul(pv[:, :], xs[:, c, 128 * tt:128 * (tt + 1)],
                                             wv[:, c, :], start=(c == 0), stop=(c == CCH - 1))
                        tv = tmp1.tile([128, 512], F32, tag="t1")
                        nc.vector.tensor_copy(tv[:, :], pv[:, :])
                        # host permuted wv cols to (d, h): contiguous 2KB move
                        nc.sync.dma_start(
                            v[:, 4 * s + tt, 0:D, :],
                            tv[:, :].rearrange("p (d h) -> p d h", h=HPC).bitcast(F32R))
                    groups.append(v_group)
                return groups

            # deferred work queues: each hp's normalize chain (PE bc matmul
            # gated on a DVE copy+reciprocal of its AV PSUM), each j-block's
            # output projection, and the later strips' projection groups are
            # emitted as PE fill inside the exp-bound attention stretches
            pending_norm = []
            oproj_queue = []
            fill_queue = []

            def pop_fill():
                if fill_queue:
                    fill_queue.pop(0)()
                elif oproj_queue:
                    emit_oproj_tt(oproj_queue.pop(0))

            def flush_norms():
                for fn in pending_norm:
                    fn()
                pending_norm.clear()

            def emit_oproj_tt(tt):
                ob = out_pool.tile([128, C], F32, tag="ob")
                for o2 in range(2):
                    po = ps_x.tile([128, 512], F32, tag="pp")
                    for c4 in range(GC // 128):
                        nc.tensor.matmul(po[:, :], avT[:, c4, 128 * tt:128 * (tt + 1)],
                                         wp[:, c4, 512 * o2:512 * (o2 + 1)],
                                         start=(c4 == 0), stop=(c4 == GC // 128 - 1))
                    nc.vector.tensor_copy(ob[:, 512 * o2:512 * (o2 + 1)], po[:, :])
                nc.sync.dma_start(out_d[128 * tt:128 * (tt + 1), :], ob[:, :])

            def emit_jblock(j):
                nkc = 4 * (j + 1)
                avs_by_hp = {}

                def emit_scores_exp(hp, i):
                    # both heads of the pair run interleaved; the pair shares
                    # one 2-bank PSUM tile so a single strided exp covers both
                    sps = ps_s.tile([128, 2, 512], F32, tag="s")
                    roff = 128 * i - 512 * j
                    diag = roff >= 0
                    roff = max(roff, 0)
                    for par in range(2):
                        h = 2 * hp + par
                        hb, hq = h % 2, h // 2
                        nc.tensor.matmul(
                            sps[:, par, roff:512],
                            kT8[32 * hb:32 * hb + 32, :, hq, 128 * i:128 * (i + 1)],
                            qtiles[j][32 * hb:32 * hb + 32, :, hq, roff:512],
                            start=True, stop=True, perf_mode=DR)
                    ptile = pt_pool.tile([128, 2, 512], F32R, tag="pt")
                    if diag:
                        # add -1e9 above the diagonal for both pars at once
                        nc.vector.tensor_tensor(
                            sps[:, :, roff:roff + 128], sps[:, :, roff:roff + 128],
                            maskneg[:, :, :], op=mybir.AluOpType.add)
                    # one exp for the par pair; cols [0:roff] are fully
                    # masked and skipped outright
                    nc.scalar.activation(ptile[:, :, roff:512], sps[:, :, roff:512],
                                         AF.Exp, scale=EXP_SCALE)
                    return ptile, roff

                def emit_av(hp, i, pts):
                    ptile, roff = pts
                    if i == 0:
                        av0 = ps_av.tile([65, 512], F32, tag="av")
                        av1 = ps_av.tile([65, 512], F32, tag="av")
                        avs_by_hp[hp] = [av0, av1]
                    for par in range(2):
                        nc.tensor.matmul(avs_by_hp[hp][par][:, roff:512],
                                         v[:, i, :, 2 * hp + par],
                                         ptile[:, par, roff:512],
                                         start=(i == 0), stop=(i == nkc - 1))

                def post_hp(hp):
                    # eager half of the normalize: the DVE copies free the AV
                    # PSUM banks for the next hp, and the bf16 reciprocal rows
                    # are ready long before the deferred PE broadcast runs
                    avs = avs_by_hp.pop(hp)
                    for par in range(2):
                        avr = avtmp_pool.tile([65, 512], F32, tag="avr")
                        nc.vector.tensor_copy(avr[:, :], avs[par][:, :])
                        rcp = avtmp_pool.tile([65, 512], BF16, tag="rcp")
                        with nc.allow_low_precision(reason="bf16 denom recip"):
                            nc.vector.reciprocal(rcp[64:65, :], avr[64:65, :])

                        def norm(avr=avr, rcp=rcp, par=par, hp=hp, j=j):
                            bc = ps_x.tile([128, 512], F32, tag="pp")
                            nc.tensor.matmul(bc[:, :], ones[64:65, :], rcp[64:65, :],
                                             start=True, stop=True)
                            avf = avtmp_pool.tile([64, 512], F32, tag="avf")
                            nc.vector.tensor_tensor(avf[:, :], avr[0:64, :],
                                                    bc[0:64, :], op=MULT)
                            nc.sync.dma_start(
                                avT[64 * par:64 * par + 64, hp, 512 * j:512 * (j + 1)],
                                avf[:, :].bitcast(F32R))
                        pending_norm.append(norm)

                # one-chunk software pipeline carried ACROSS hp boundaries:
                # the next group's scores always sit ahead of the previous
                # group's AV matmuls in the PE stream, so the ACT exp stream
                # never waits at an hp seam. Deferred normalize and out-proj
                # tiles slot in as PE fill during the exp-bound stretches.
                prev = None
                # fill slots are budgeted against each block's ACT slack:
                # jb0/jb1 have little (exp stream barely ahead), jb2 some,
                # jb3 the most (its exps dominate while PE has nothing left)
                def fill_slot(i):
                    if j == 0:
                        return i == 3
                    if j in (1, 2):
                        return i >= 3 and i % 2 == 1
                    return i in (3, 7, 11, 15)
                for hp in range(HPC // 2):
                    for i in range(nkc):
                        cur = (hp, i, emit_scores_exp(hp, i))
                        if i == (1 if j == 0 else 2):
                            flush_norms()
                        elif fill_slot(i):
                            pop_fill()
                        if prev is not None:
                            php, pi, pts = prev
                            emit_av(php, pi, pts)
                            if pi == nkc - 1:
                                post_hp(php)
                        prev = cur
                php, pi, pts = prev
                emit_av(php, pi, pts)
                post_hp(php)

                oproj_queue.extend(range(4 * j, 4 * (j + 1)))

            # strips 0/1 run ahead of the attention blocks so the ACT exp
            # stream (the binding engine of the attention stretches) starts
            # early; strips 2/3 issue their loads between blocks but their
            # projection matmuls are spread as PE fill inside the previous
            # j-block, so ACT never starves behind a strip
            xs8_0 = emit_strip_loads_a(0)
            # strip-0 q/k compute runs as a block; its eviction DMAs hit the
            # DMA device before the heavy V loads are even issued
            for g in qk_compute_groups(0, xs8_0):
                g()
            xs_0 = emit_strip_loads_b(0)
            for g in v_compute_groups(0, xs_0):
                g()
            xs8_1 = emit_strip_loads_a(1)
            xs_1 = emit_strip_loads_b(1)
            qk1 = qk_compute_groups(1, xs8_1)
            fill_queue.extend(qk1[0:4])
            emit_jblock(0)
            while fill_queue:
                fill_queue.pop(0)()
            for g in qk1[4:8]:
                g()
            xs8_2 = emit_strip_loads_a(2)
            xs_2 = emit_strip_loads_b(2)
            fill_queue.extend(v_compute_groups(1, xs_1) + qk_compute_groups(2, xs8_2))
            emit_jblock(1)
            while fill_queue:
                fill_queue.pop(0)()
            xs8_3 = emit_strip_loads_a(3)
            xs_3 = emit_strip_loads_b(3)
            fill_queue.extend(v_compute_groups(2, xs_2) + qk_compute_groups(3, xs8_3)
                              + v_compute_groups(3, xs_3))
            emit_jblock(2)
            while fill_queue:
                fill_queue.pop(0)()
            emit_jblock(3)
            flush_norms()
            while fill_queue:
                fill_queue.pop(0)()
            while oproj_queue:
                emit_oproj_tt(oproj_queue.pop(0))
    _split_excess_waits(nc)
    return nc


def _get_program():
    global _PROGRAM
    if _PROGRAM is None:
        _PROGRAM = _build_program()
    return _PROGRAM


def _make_in_maps(x, Wk, Wq, Wv, Wp):
    import ml_dtypes
    f8 = ml_dtypes.float8_e4m3
    x = np.asarray(x, dtype=np.float32)
    Wk = np.asarray(Wk, dtype=np.float32)
    Wq = np.asarray(Wq, dtype=np.float32)
    Wv = np.asarray(Wv, dtype=np.float32)
    Wp = np.asarray(Wp, dtype=np.float32)
    # fp8 weight column permutation: col 128*o+64*sh+32*hh+d32 holds original
    # out-channel (2*o+hh)*64 + 32*sh + d32, so the q/k eviction DMA is a pure
    # partition-offset move into the [32*(h%4)+d32, d//32, h//4, t] layout
    perm = np.empty(GC, np.int64)
    for o in range(4):
        for sh in range(2):
            for hh in range(2):
                base = 128 * o + 64 * sh + 32 * hh
                perm[base:base + 32] = (2 * o + hh) * 64 + 32 * sh + np.arange(32)
    # wv column permutation: col d*8+h holds original out-channel h*64+d, so
    # the v eviction writes d-major contiguous runs
    perm_v = np.empty(GC, np.int64)
    for d in range(D):
        perm_v[d * HPC:(d + 1) * HPC] = np.arange(HPC) * D + d
    in_maps = []
    for core in range(8):
        b, g = core // GROUPS, core % GROUPS
        rows = slice(GC * g, GC * (g + 1))
        xT = np.ascontiguousarray(x[b].T)
        in_maps.append({
            "xT": xT,                                           # [C, T]
            "xT8": np.ascontiguousarray(xT.astype(f8)),
            "wq8": np.ascontiguousarray((WSCALE * Wq[rows, :][perm, :].T).astype(f8)),
            "wk8": np.ascontiguousarray((WSCALE * Wk[rows, :][perm, :].T).astype(f8)),
            "wvT": np.ascontiguousarray(Wv[rows, :][perm_v, :].T),
            "wpT": np.ascontiguousarray(Wp[:, rows].T),         # [GC, C]
        })
    return in_maps


def run(x, Wk, Wq, Wv, Wp, bp, trace=False, **spmd_kwargs):
    nc = _get_program()
    in_maps = _make_in_maps(x, Wk, Wq, Wv, Wp)
    res = run_bass_kernel_spmd(nc, in_maps, list(range(8)), trace=trace, **spmd_kwargs)
    bp = np.asarray(bp, dtype=np.float32)
    out = np.empty((B, T, C), dtype=np.float32)
    for b in range(B):
        out[b] = res.results[GROUPS * b]["outp"] + res.results[GROUPS * b + 1]["outp"] + bp
    return out, res


def kernel(x, Wk, Wq, Wv, Wp, bp):
    out, _ = run(x, Wk, Wq, Wv, Wp, bp)
    return out



# revision 60
# speedup vs baseline: 1.5189x; 1.0693x over previous
"""Multi-head causal self-attention (B=4, T=2048, C=1024, 16 heads) on 8 trn2 cores.

Sharding: data-parallel over batch (4) x tensor-parallel over heads (2 groups of 8).
Core m handles batch m//2, head group m%2. Host pre-transposes/pre-quantizes the
operands so every on-device matmul consumes natural layouts; the output projection
partial sums are pair-reduced on host (+bias).

Per-core pipeline:
  Q,K projections: fp8e4m3 DoubleRow matmuls (host supplies x and 32*W in fp8,
    weight columns permuted so the fp8 q/k evictions are pure partition moves).
  V projection + out-projection: fp32r (precision-critical paths).
  scores^T[k,q] per head: fp8 DoubleRow off the [32*(h%2)+d%32, d//32, h//2, t]
    q/k layout (exp scale absorbs the 32*32 weight scaling).
  p = exp(scores): ACT engine, one strided exp per head-pair over a 2-bank PSUM
    tile; causal mask added in PSUM by a strided DVE add on diagonal chunks.
  [AV^T | denom] = [v|1]^T @ p: fp32r; the ones row of v (d-major layout via a
    host wv column permutation) gives the softmax denominators for free.
  normalize: DVE bf16 reciprocal of the denom row, K=1 ones-matmul broadcast,
    DVE multiply; deferred into the next head-pair's chunk stream as PE fill.
  schedule: projection strips, attention j-blocks, normalize chains, and the
    out-projection are interleaved via fill queues so the ACT-bound exp stream
    and the PE stream overlap; heavy loads are split/ordered for DMA latency.
"""

import numpy as np

import concourse.bass as bass
import concourse.mybir as mybir
import concourse.tile as tile
from concourse.bass_utils import run_bass_kernel_spmd

F32 = mybir.dt.float32
F32R = mybir.dt.float32r
F8 = mybir.dt.float8e4
BF16 = mybir.dt.bfloat16
DR = mybir.MatmulPerfMode.DoubleRow
AF = mybir.ActivationFunctionType
MULT = mybir.AluOpType.mult
WSCALE = 32.0               # host scales Wq/Wk by 32 so fp8 stays out of subnormals
EXP_SCALE = 0.125 / (WSCALE * WSCALE)

B, T, C = 4, 2048, 1024
HEADS, D = 16, 64
GROUPS = 2                  # head groups (tensor parallel)
HPC = HEADS // GROUPS       # heads per core = 8
GC = HPC * D                # group channel width = 512
NKC = T // 128              # Tk chunks = 16
NJ = T // 512               # Tq tiles = 4
CCH = C // 128              # contraction chunks = 8
NSTRIP = T // 512           # phase-1 t strips = 4

_PROGRAM = None


def _patch_drain_chunking():
    """The axon walrus build rejects instructions with >~4 sem waits; Tile's
    kernel-tail drain waits on every live semaphore at once. Split it into a
    chain of drains with <=2 waits each."""
    from bass_rust import VectorClock, ScopedClock

    if getattr(tile.TileContext, "_drain_chunk_patched", False):
        return

    def _drain_and_barrier(self, tick_clock, wait_clock):
        gc_vec = list(tick_clock.global_clock)
        nz = [i for i, t in enumerate(gc_vec) if t > 0]
        CHUNK = 1
        for k in range(0, len(nz), CHUNK):
            keep = set(nz[k:k + CHUNK])
            partial = [gc_vec[i] if i in keep else 0 for i in range(len(gc_vec))]
            d = self.nc.sync.drain()
            wait_clock.add_sem_waits(d.ins, ScopedClock({None: VectorClock(partial)}))
        self.nc.all_engine_barrier()
        assert self.sems is not None
        popped = self.nc._tile_sem_poison_stack.pop()
        assert popped is self._sem_poison
        self.nc.clear_and_free_semaphores(list(self.sems.allocated().values()))
        self.nc.all_engine_barrier()

    tile.TileContext._drain_and_barrier = _drain_and_barrier
    tile.TileContext._drain_chunk_patched = True


def _split_excess_waits(nc, maxw=1, maxw_other=None):
    """Walrus rejects instructions carrying more than ~1 sem wait (proven for
    PE matmul S3_LW and the SP drain at 5). Move excess waits onto same-engine
    NoOps inserted immediately before the instruction (engine streams execute
    in bb order, so semantics are preserved). maxw_other, if set, applies to
    non-PE engines."""
    from bass_rust import InstNoOp

    ctr = 0
    for f in nc.m.functions:
        for bb in f.blocks:
            new_insts = []
            for inst in bb.instructions:
                si = inst.sync_info
                waits = list(si.on_wait) if si and si.on_wait else []
                lim = maxw
                if maxw_other is not None and str(inst.engine) != 'EngineType.PE':
                    lim = maxw_other
                maxw_eff = lim
                if len(waits) > maxw_eff:
                    head, rest = waits[:-maxw_eff], waits[-maxw_eff:]
                    for k in range(0, len(head), maxw_eff):
                        ctr += 1
                        new_insts.append(InstNoOp(
                            name=f"waitnop_{ctr}",
                            engine=inst.engine,
                            sync_info=mybir.SyncInfo(
                                on_wait=head[k:k + maxw_eff], on_update=[]),
                        ))
                    inst.sync_info = mybir.SyncInfo(on_wait=rest, on_update=si.on_update)
                new_insts.append(inst)
            bb.instructions = new_insts
    return ctr


def _build_program():
    _patch_drain_chunking()
    nc = bass.Bass()

    xT_d = nc.declare_dram_parameter("xT", [C, T], F32R, isOutput=False)
    xT8_d = nc.declare_dram_parameter("xT8", [C, T], F8, isOutput=False)
    wq8_d = nc.declare_dram_parameter("wq8", [C, GC], F8, isOutput=False)
    wk8_d = nc.declare_dram_parameter("wk8", [C, GC], F8, isOutput=False)
    wv_d = nc.declare_dram_parameter("wvT", [C, GC], F32R, isOutput=False)
    wp_d = nc.declare_dram_parameter("wpT", [GC, C], F32R, isOutput=False)
    out_d = nc.declare_dram_parameter("outp", [T, C], F32, isOutput=True)

    from contextlib import ExitStack

    with tile.TileContext(nc) as tc, ExitStack() as stack:
        cpool = stack.enter_context(tc.tile_pool(name="const", bufs=1))
        qkv_pool = stack.enter_context(tc.tile_pool(name="qkv", bufs=1))

        # additive causal mask: 0 where q >= k, -1e9 where q < k (exp -> 0),
        # replicated on both par slots so one strided DVE add masks the pair
        maskneg = cpool.tile([128, 2, 128], F32)
        nc.gpsimd.memset(maskneg[:, :, :], 0.0)
        for s in range(2):
            nc.gpsimd.affine_select(
                out=maskneg[:, s, :], in_=maskneg[:, s, :],
                compare_op=mybir.AluOpType.is_ge, fill=-1e9, base=0,
                pattern=[[1, 128]], channel_multiplier=-1,
            )
        # DVE cannot encode f32r, so f32r tiles are written by ACT/DMA only.
        # ones row is bf16 so the K=1 broadcast matmul can take a DVE-written
        # bf16 rhs (DVE cannot write f32r; verifier demands rounded operands)
        ones = cpool.tile([128, 128], BF16)
        nc.scalar.activation(ones[64:65, :], ones[64:65, :], AF.Copy, scale=0.0, bias=1.0)

        # fp8 q/k for DoubleRow scores: partition = 32*(h%2) + d%32 (PE operand
        # base partition must be 0/32/64), free = [d//32 (stacked contraction
        # half), h//2, t]
        kT8 = qkv_pool.tile([64, 2, HPC // 2, T], F8)
        # v padded with a ones row per head: [t-chunk, d, head] (d-major via
        # host wv column permutation so the eviction DMA moves 2KB-contiguous
        # runs); row 64 = ones for the softmax denominators
        v = qkv_pool.tile([128, NKC, D + 1, HPC], F32R)
        nc.scalar.activation(v[:, :, D, :], v[:, :, D, :],
                             AF.Copy, scale=0.0, bias=1.0)

        avT = qkv_pool.tile([128, HPC // 2, T], F32R)
        wp = qkv_pool.tile([128, GC // 128, C], F32R)

        # q lives in per-strip tiles (jb j only reads strip j's queries)
        # Interleaved schedule: phase-1 strip s immediately followed by the
        # attention j-block j=s (its queries live in strip s; its keys/values
        # in strips 0..s), so the SP-bound projection DMAs and the ACT-bound
        # exp stream overlap instead of running as separate phases.
        # PSUM budget (8 banks): ps_s 2x2 + ps_av 2 + ps_x 2 (shared by the
        # qkv projection accumulators, the denom broadcast, and out-proj).
        with tc.tile_pool(name="w1", bufs=1) as wpool, \
             tc.tile_pool(name="xs", bufs=2) as xpool, \
             tc.tile_pool(name="xs8", bufs=2) as x8pool, \
             tc.tile_pool(name="stg", bufs=1) as stgpool, \
             tc.tile_pool(name="qst", bufs=2) as qpool, \
             tc.tile_pool(name="tmp1", bufs=2) as tmp1, \
             tc.tile_pool(name="pt", bufs=3) as pt_pool, \
             tc.tile_pool(name="avtmp", bufs=2) as avtmp_pool, \
             tc.tile_pool(name="ob", bufs=2) as out_pool, \
             tc.tile_pool(name="ps_s", bufs=2, space="PSUM") as ps_s, \
             tc.tile_pool(name="ps_av", bufs=2, space="PSUM") as ps_av, \
             tc.tile_pool(name="ps_x", bufs=2, space="PSUM") as ps_x:
            wq8 = wpool.tile([128, CCH, GC], F8)
            wk8 = wpool.tile([128, CCH, GC], F8)
            wv = wpool.tile([128, CCH, GC], F32R)

            def emit_strip_loads_a(s):
                # latency-critical small fp8 loads: x strip + (once) q/k weights
                xs8 = x8pool.tile([128, CCH, 512], F8)
                nc.sync.dma_start(
                    xs8[:, :, :],
                    xT8_d[:, 512 * s:512 * (s + 1)].rearrange("(c p) t -> p c t", p=128))
                if s == 0:
                    for w_sb, w_d in ((wq8, wq8_d), (wk8, wk8_d)):
                        nc.sync.dma_start(w_sb[:, :, :],
                                          w_d[:, :].rearrange("(c p) o -> p c o", p=128))
                return xs8

            def emit_strip_loads_b(s):
                # heavy f32r loads for the V path; emitted after the q/k
                # eviction DMAs of the covering strip so those small transfers
                # win the DMA device first
                xs = xpool.tile([128, CCH, 512], F32R)
                cs0 = slice(0, 4)
                nc.sync.dma_start(
                    xs[:, cs0, :],
                    xT_d[0:512, 512 * s:512 * (s + 1)].rearrange("(c p) t -> p c t", p=128))
                if s == 0:
                    nc.sync.dma_start(wv[:, :, :],
                                      wv_d[:, :].rearrange("(c p) o -> p c o", p=128))
                cs1 = slice(4, 8)
                nc.sync.dma_start(
                    xs[:, cs1, :],
                    xT_d[512:1024, 512 * s:512 * (s + 1)].rearrange("(c p) t -> p c t", p=128))
                if s == 2:
                    # wp is first consumed by the deferred out-proj of jb0
                    # (inside jb1)
                    nc.sync.dma_start(wp[:, :, :],
                                      wp_d[:, :].rearrange("(c p) o -> p c o", p=128))
                return xs

            qtiles = {}

            def qk_compute_groups(s, xs8):
                # one closure per q/k projection tile
                groups = []
                stages = {}
                for w_sb, stag in ((wq8, "sq"), (wk8, "sk")):
                    for o in range(HPC // 2):
                        def qk_group(w_sb=w_sb, stag=stag, o=o):
                            if stag not in stages:
                                stage_t = stgpool.tile([128, HPC // 2, 512], F8, tag=stag)
                                stages[stag] = stage_t
                            stage = stages[stag]
                            pq = ps_x.tile([128, 512], F32, tag="pp")
                            for c2 in range(CCH // 2):
                                nc.tensor.matmul(
                                    pq[:, :],
                                    w_sb[:, 2 * c2:2 * c2 + 2, 128 * o:128 * (o + 1)],
                                    xs8[:, 2 * c2:2 * c2 + 2, :],
                                    start=(c2 == 0), stop=(c2 == CCH // 2 - 1),
                                    perf_mode=DR)
                            # fp8 eviction on DVE (keeps ACT free for exps)
                            nc.vector.tensor_copy(stage[:, o, :], pq[:, :])
                            if o == HPC // 2 - 1:
                                # staging row 64*sh+32*hh+d32 at slot o holds
                                # head 2o+hh, d=32*sh+d32 (host permuted W
                                # cols); move to partition 32*hh+d32, [sh, o]
                                if stag == "sq":
                                    qt = qpool.tile([64, 2, HPC // 2, 512], F8, tag="qT8")
                                    qtiles[s] = qt
                                    dst_aps = [qt[:, sh, :, :] for sh in range(2)]
                                else:
                                    dst_aps = [kT8[:, sh, :, 512 * s:512 * (s + 1)]
                                               for sh in range(2)]
                                for sh in range(2):
                                    nc.sync.dma_start(
                                        dst_aps[sh],
                                        stage[64 * sh:64 * sh + 64, :, :])
                        groups.append(qk_group)
                return groups

            def v_compute_groups(s, xs):
                groups = []
                for tt in range(4):
                    def v_group(tt=tt):
                        pv = ps_x.tile([128, 512], F32, tag="pp")
                        for c in range(CCH):
                            nc.tensor.matmul(pv[:, :], xs[:, c, 128 * tt:128 * (tt + 1)],
                                             wv[:, c, :], start=(c == 0), stop=(c == CCH - 1))
                        tv = tmp1.tile([128, 512], F32, tag="t1")
                        nc.vector.tensor_copy(tv[:, :], pv[:, :])
                        # host permuted wv cols to (d, h): contiguous 2KB move
                        nc.sync.dma_start(
                            v[:, 4 * s + tt, 0:D, :],
                            tv[:, :].rearrange("p (d h) -> p d h", h=HPC).bitcast(F32R))
                    groups.append(v_group)
                return groups

            # deferred work queues: each hp's normalize chain (PE bc matmul
            # gated on a DVE copy+reciprocal of its AV PSUM), each j-block's
            # output projection, and the later strips' projection groups are
            # emitted as PE fill inside the exp-bound attention stretches
            pending_norm = []
            oproj_queue = []
            fill_queue = []

            def pop_fill():
                if fill_queue:
                    fill_queue.pop(0)()
                elif oproj_queue:
                    emit_oproj_tt(oproj_queue.pop(0))

            def flush_norms():
                for fn in pending_norm:
                    fn()
                pending_norm.clear()

            def emit_oproj_tt(tt):
                ob = out_pool.tile([128, C], F32, tag="ob")
                for o2 in range(2):
                    po = ps_x.tile([128, 512], F32, tag="pp")
                    for c4 in range(GC // 128):
                        nc.tensor.matmul(po[:, :], avT[:, c4, 128 * tt:128 * (tt + 1)],
                                         wp[:, c4, 512 * o2:512 * (o2 + 1)],
                                         start=(c4 == 0), stop=(c4 == GC // 128 - 1))
                    nc.vector.tensor_copy(ob[:, 512 * o2:512 * (o2 + 1)], po[:, :])
                nc.sync.dma_start(out_d[128 * tt:128 * (tt + 1), :], ob[:, :])

            def emit_jblock(j):
                nkc = 4 * (j + 1)
                avs_by_hp = {}

                def emit_scores_exp(hp, i):
                    # both heads of the pair run interleaved; the pair shares
                    # one 2-bank PSUM tile so a single strided exp covers both
                    sps = ps_s.tile([128, 2, 512], F32, tag="s")
                    roff = 128 * i - 512 * j
                    diag = roff >= 0
                    roff = max(roff, 0)
                    for par in range(2):
                        h = 2 * hp + par
                        hb, hq = h % 2, h // 2
                        nc.tensor.matmul(
                            sps[:, par, roff:512],
                            kT8[32 * hb:32 * hb + 32, :, hq, 128 * i:128 * (i + 1)],
                            qtiles[j][32 * hb:32 * hb + 32, :, hq, roff:512],
                            start=True, stop=True, perf_mode=DR)
                    ptile = pt_pool.tile([128, 2, 512], F32R, tag="pt")
                    if diag:
                        # add -1e9 above the diagonal for both pars at once
                        nc.vector.tensor_tensor(
                            sps[:, :, roff:roff + 128], sps[:, :, roff:roff + 128],
                            maskneg[:, :, :], op=mybir.AluOpType.add)
                    # one exp for the par pair; cols [0:roff] are fully
                    # masked and skipped outright
                    nc.scalar.activation(ptile[:, :, roff:512], sps[:, :, roff:512],
                                         AF.Exp, scale=EXP_SCALE)
                    return ptile, roff

                def emit_av(hp, i, pts):
                    ptile, roff = pts
                    if i == 0:
                        av0 = ps_av.tile([65, 512], F32, tag="av")
                        av1 = ps_av.tile([65, 512], F32, tag="av")
                        avs_by_hp[hp] = [av0, av1]
                    for par in range(2):
                        nc.tensor.matmul(avs_by_hp[hp][par][:, roff:512],
                                         v[:, i, :, 2 * hp + par],
                                         ptile[:, par, roff:512],
                                         start=(i == 0), stop=(i == nkc - 1))

                def post_hp(hp):
                    # eager half of the normalize: the DVE copies free the AV
                    # PSUM banks for the next hp, and the bf16 reciprocal rows
                    # are ready long before the deferred PE broadcast runs
                    avs = avs_by_hp.pop(hp)
                    for par in range(2):
                        avr = avtmp_pool.tile([65, 512], F32, tag="avr")
                        nc.vector.tensor_copy(avr[:, :], avs[par][:, :])
                        rcp = avtmp_pool.tile([65, 512], BF16, tag="rcp")
                        with nc.allow_low_precision(reason="bf16 denom recip"):
                            nc.vector.reciprocal(rcp[64:65, :], avr[64:65, :])

                        def norm(avr=avr, rcp=rcp, par=par, hp=hp, j=j):
                            bc = ps_x.tile([128, 512], F32, tag="pp")
                            nc.tensor.matmul(bc[:, :], ones[64:65, :], rcp[64:65, :],
                                             start=True, stop=True)
                            avf = avtmp_pool.tile([64, 512], F32, tag="avf")
                            nc.vector.tensor_tensor(avf[:, :], avr[0:64, :],
                                                    bc[0:64, :], op=MULT)
                            nc.sync.dma_start(
                                avT[64 * par:64 * par + 64, hp, 512 * j:512 * (j + 1)],
                                avf[:, :].bitcast(F32R))
                        pending_norm.append(norm)

                # one-chunk software pipeline carried ACROSS hp boundaries:
                # the next group's scores always sit ahead of the previous
                # group's AV matmuls in the PE stream, so the ACT exp stream
                # never waits at an hp seam. Deferred normalize and out-proj
                # tiles slot in as PE fill during the exp-bound stretches.
                prev = None
                # fill slots are budgeted against each block's ACT slack:
                # jb0/jb1 have little (exp stream barely ahead), jb2 some,
                # jb3 the most (its exps dominate while PE has nothing left)
                def fill_slot(i):
                    if j == 0:
                        return i == 3
                    if j in (1, 2):
                        return i >= 3 and i % 2 == 1
                    return i in (3, 7, 11, 15)
                for hp in range(HPC // 2):
                    for i in range(nkc):
                        cur = (hp, i, emit_scores_exp(hp, i))
                        if i == (2 if j == 0 else nkc - 1):
                            flush_norms()
                        elif fill_slot(i):
                            pop_fill()
                        if prev is not None:
                            php, pi, pts = prev
                            emit_av(php, pi, pts)
                            if pi == nkc - 1:
                                post_hp(php)
                        prev = cur
                php, pi, pts = prev
                emit_av(php, pi, pts)
                post_hp(php)

                oproj_queue.extend(range(4 * j, 4 * (j + 1)))

            # strips 0/1 run ahead of the attention blocks so the ACT exp
            # stream (the binding engine of the attention stretches) starts
            # early; strips 2/3 issue their loads between blocks but their
            # projection matmuls are spread as PE fill inside the previous
            # j-block, so ACT never starves behind a strip
            xs8_0 = emit_strip_loads_a(0)
            # strip-0 q/k compute runs as a block; its eviction DMAs hit the
            # DMA device before the heavy V loads are even issued
            for g in qk_compute_groups(0, xs8_0):
                g()
            xs_0 = emit_strip_loads_b(0)
            for g in v_compute_groups(0, xs_0):
                g()
            xs8_1 = emit_strip_loads_a(1)
            xs_1 = emit_strip_loads_b(1)
            qk1 = qk_compute_groups(1, xs8_1)
            fill_queue.extend(qk1[0:4])
            emit_jblock(0)
            while fill_queue:
                fill_queue.pop(0)()
            for g in qk1[4:8]:
                g()
            xs8_2 = emit_strip_loads_a(2)
            xs_2 = emit_strip_loads_b(2)
            fill_queue.extend(v_compute_groups(1, xs_1) + qk_compute_groups(2, xs8_2))
            emit_jblock(1)
            while fill_queue:
                fill_queue.pop(0)()
            xs8_3 = emit_strip_loads_a(3)
            xs_3 = emit_strip_loads_b(3)
            fill_queue.extend(v_compute_groups(2, xs_2) + qk_compute_groups(3, xs8_3)
                              + v_compute_groups(3, xs_3))
            emit_jblock(2)
            while fill_queue:
                fill_queue.pop(0)()
            emit_jblock(3)
            flush_norms()
            while fill_queue:
                fill_queue.pop(0)()
            while oproj_queue:
                emit_oproj_tt(oproj_queue.pop(0))
    _split_excess_waits(nc)
    return nc


def _get_program():
    global _PROGRAM
    if _PROGRAM is None:
        _PROGRAM = _build_program()
    return _PROGRAM


def _make_in_maps(x, Wk, Wq, Wv, Wp):
    import ml_dtypes
    f8 = ml_dtypes.float8_e4m3
    x = np.asarray(x, dtype=np.float32)
    Wk = np.asarray(Wk, dtype=np.float32)
    Wq = np.asarray(Wq, dtype=np.float32)
    Wv = np.asarray(Wv, dtype=np.float32)
    Wp = np.asarray(Wp, dtype=np.float32)
    # fp8 weight column permutation: col 128*o+64*sh+32*hh+d32 holds original
    # out-channel (2*o+hh)*64 + 32*sh + d32, so the q/k eviction DMA is a pure
    # partition-offset move into the [32*(h%4)+d32, d//32, h//4, t] layout
    perm = np.empty(GC, np.int64)
    for o in range(4):
        for sh in range(2):
            for hh in range(2):
                base = 128 * o + 64 * sh + 32 * hh
                perm[base:base + 32] = (2 * o + hh) * 64 + 32 * sh + np.arange(32)
    # wv column permutation: col d*8+h holds original out-channel h*64+d, so
    # the v eviction writes d-major contiguous runs
    perm_v = np.empty(GC, np.int64)
    for d in range(D):
        perm_v[d * HPC:(d + 1) * HPC] = np.arange(HPC) * D + d
    in_maps = []
    for core in range(8):
        b, g = core // GROUPS, core % GROUPS
        rows = slice(GC * g, GC * (g + 1))
        xT = np.ascontiguousarray(x[b].T)
        in_maps.append({
            "xT": xT,                                           # [C, T]
            "xT8": np.ascontiguousarray(xT.astype(f8)),
            "wq8": np.ascontiguousarray((WSCALE * Wq[rows, :][perm, :].T).astype(f8)),
            "wk8": np.ascontiguousarray((WSCALE * Wk[rows, :][perm, :].T).astype(f8)),
            "wvT": np.ascontiguousarray(Wv[rows, :][perm_v, :].T),
            "wpT": np.ascontiguousarray(Wp[:, rows].T),         # [GC, C]
        })
    return in_maps


def run(x, Wk, Wq, Wv, Wp, bp, trace=False, **spmd_kwargs):
    nc = _get_program()
    in_maps = _make_in_maps(x, Wk, Wq, Wv, Wp)
    res = run_bass_kernel_spmd(nc, in_maps, list(range(8)), trace=trace, **spmd_kwargs)
    bp = np.asarray(bp, dtype=np.float32)
    out = np.empty((B, T, C), dtype=np.float32)
    for b in range(B):
        out[b] = res.results[GROUPS * b]["outp"] + res.results[GROUPS * b + 1]["outp"] + bp
    return out, res


def kernel(x, Wk, Wq, Wv, Wp, bp):
    out, _ = run(x, Wk, Wq, Wv, Wp, bp)
    return out

